# revision 1
# baseline (speedup 1.0000x reference)
"""DeepSeek-V3 MLA forward (B=1, S=2048, D=4096, H=32) on 8 TRN2 NeuronCores.

v2: sequence-sharded low-rank a-projections + in-kernel collectives.

Structure (per core c of 8):
  * Core c owns 256 seq columns: 64-col blocks {512r + 64c : r in 0..3}.
  * Phase A (local): a-projections computed only for the owned columns in
    fp8e4 DoubleRow 3-term hi/lo (error ~1e-3, 0.5 cyc/row). RMS scales
    applied locally; weights pre-scaled by 64/512 into fp8 range with the
    inverse folded into rms (scale-invariant), rope tables, or evacuation
    scales.
  * Normalized ckv + rope key AllGather'ed (2.4MB, ~75us, overlapped).
  * Phase Q (local): q b-projection for ALL 32 heads on owned columns,
    fp8 DoubleRow 3-term, weights streamed from DRAM; rope applied locally.
  * 4 chunked AllToAlls redistribute q^T feature-major to head-owners
    (core d owns heads 4d..4d+3), one per 512-col query tile, pipelined
    against attention.
  * Phase B: kv b-projection from gathered ckv (bf16).
  * Phase C: causal attention with transposed scores + fp8 DoubleRow
    3-term out-projection; host sums the 8 partial out-projections.

Engine budget: PE ~360us busy; evacuations spread DVE/ACT; DMAs batched and
issued from producer engines to avoid SP sequencer head-of-line blocking.
"""

from dataclasses import dataclass

import ml_dtypes
import numpy as np

import concourse.bass as bass
import concourse.mybir as mybir
import concourse.tile as tile
from concourse import bacc
from concourse.bass_utils import run_bass_kernel_spmd

F32 = mybir.dt.float32
F32R = mybir.dt.float32r
BF16 = mybir.dt.bfloat16
FP8 = mybir.dt.float8e4
AF = mybir.ActivationFunctionType
DR = mybir.MatmulPerfMode.DoubleRow
BF16NP = ml_dtypes.bfloat16
E4M3 = ml_dtypes.float8_e4m3

N_CORES = 8
GRP = [[0, 1, 2, 3, 4, 5, 6, 7]]
EPS = 1e-6
THETA = 10000.0

SA = 64.0     # a-proj weight prescale (folded out via rms / rope tables)
SQ = 512.0    # q b-proj weight prescale (folded out at evacuation)
SWO = 64.0    # out-proj weight prescale (folded out at evacuation)


@dataclass(frozen=True)
class Cfg:
    S: int = 2048
    D: int = 4096
    QR: int = 1536
    KVR: int = 512
    H: int = 32
    HPC: int = 4
    NOPE: int = 128
    ROPE: int = 64
    VD: int = 128

    @property
    def DP(self):          # 128x2 contraction pairs in D
        return self.D // 256

    @property
    def QP(self):          # pairs in q lora rank
        return self.QR // 256

    @property
    def QRCH(self):
        return self.QR // 128

    @property
    def KVCH(self):
        return self.KVR // 128

    @property
    def AM(self):          # a-proj out chunks: 12 qa + 4 ckv + 1 rope(pad)
        return self.QRCH + self.KVCH + 1

    @property
    def LOC(self):         # owned columns per core
        return self.S // N_CORES

    @property
    def NQT(self):
        return self.S // 512

    @property
    def NKI(self):
        return self.S // 128

    @property
    def QCH(self):         # q b-proj out chunks: 32 nope + 16 rope
        return self.H + self.H * self.ROPE // 128

    @property
    def GR(self):          # gathered rows: 4*128 ckv + 64 rope key
        return self.KVCH * 128 + self.ROPE


FULL = Cfg()


# --------------------------------------------------------------------------
# host-side input preparation
# --------------------------------------------------------------------------

def _rope_perm(rope):
    return np.concatenate([np.arange(0, rope, 2), np.arange(1, rope, 2)])


def _split8(x):
    hi = x.astype(E4M3)
    lo = (x - hi.astype(np.float32)).astype(E4M3)
    return hi, lo


def prep_inputs(cfg, hidden_states, Wq_a, q_a_ln_w, Wq_b, Wkv_a, kv_a_ln_w,
                Wkv_b, Wo):
    c = cfg
    hs = np.asarray(hidden_states, np.float32).reshape(c.S, c.D)
    Wq_a = np.asarray(Wq_a, np.float32)
    Wq_b = np.asarray(Wq_b, np.float32)
    Wkv_a = np.asarray(Wkv_a, np.float32)
    Wkv_b = np.asarray(Wkv_b, np.float32)
    Wo = np.asarray(Wo, np.float32)
    q_a_ln_w = np.asarray(q_a_ln_w, np.float32)
    kv_a_ln_w = np.asarray(kv_a_ln_w, np.float32)

    nope, rope, vd = c.NOPE, c.ROPE, c.VD
    qd = nope + rope

    # combined a-proj weight (x SA), rope cols permuted, padded to 17*128;
    # hi/lo planes packed per m-chunk: [AM, 128, 2, DP, 2, 128]
    perm_a = _rope_perm(rope)
    Wkv_a_p = np.concatenate(
        [Wkv_a[:, :c.KVR], Wkv_a[:, c.KVR:][:, perm_a]], axis=1)
    wa = np.concatenate([Wq_a, Wkv_a_p], axis=1) * SA      # [D, 2112]
    wa = np.pad(wa, ((0, 0), (0, c.AM * 128 - wa.shape[1])))
    wah_np, wal_np = _split8(wa)

    def _wa_prep(w8):                                      # [AM,128,DP,2,128]
        t = w8.reshape(c.DP, 2, 128, c.AM, 128).transpose(3, 2, 0, 1, 4)
        return np.ascontiguousarray(t)
    wa_hl = np.ascontiguousarray(np.stack(
        [_wa_prep(wah_np), _wa_prep(wal_np)], axis=2))
    # -> [AM, 128, 2, DP, 2, 128]

    # q b-proj weights (x SQ), ln + 1/sqrt(qd) + rope perm folded;
    # out-chunk order: 0..31 = nope of head o; 32+j = rope of heads 2j,2j+1;
    # grouped 4 chunks per DMA: [12, 128, 4, 2, QP, 2, 128]
    scale = qd ** (-0.5)
    wqb_all = (Wq_b * q_a_ln_w[:, None]).reshape(c.QR, c.H, qd) * scale * SQ
    perm = _rope_perm(rope)
    wqb_nope = wqb_all[:, :, :nope]
    wqb_rope = wqb_all[:, :, nope:][:, :, perm]
    cols = [wqb_nope[:, h, :] for h in range(c.H)]
    for j in range(c.H // 2):
        cols.append(np.concatenate(
            [wqb_rope[:, 2 * j, :], wqb_rope[:, 2 * j + 1, :]], axis=1))
    wqb = np.stack(cols, axis=0)                            # [48, QR, 128]
    wqbh_np, wqbl_np = _split8(wqb)

    def _wqb_prep(w8):                                      # [48,128,QP,2,128]
        t = w8.reshape(c.QCH, c.QP, 2, 128, 128).transpose(0, 3, 1, 2, 4)
        return np.ascontiguousarray(t)
    wqb_hl = np.stack([_wqb_prep(wqbh_np), _wqb_prep(wqbl_np)], axis=2)
    # [48, 128, 2, QP, 2, 128] -> [12, 128, 4, 2, QP, 2, 128]
    wqb_hl = np.ascontiguousarray(
        wqb_hl.reshape(12, 4, 128, 2, c.QP, 2, 128).transpose(
            0, 2, 1, 3, 4, 5, 6))

    # kv b-proj weights (bf16), ln folded; packed [128, KVCH, 2, 512]
    wkv_all = (Wkv_b * kv_a_ln_w[:, None]).reshape(c.KVR, c.H, nope + vd)

    # rope tables, feature-major [128, S]
    inv_freq = 1.0 / (THETA ** (np.arange(0, rope, 2, np.float32) / rope))
    freqs = np.outer(np.arange(c.S, dtype=np.float32), inv_freq)
    cosT = np.tile(np.cos(freqs).T, (4, 1)).astype(np.float32)
    sinT = np.tile(np.sin(freqs).T, (4, 1)).astype(np.float32)

    R = np.zeros((128, 128), np.float32)
    for blk in (0, 64):
        for i in range(32):
            R[blk + i, blk + i + 32] = -1.0
            R[blk + i + 32, blk + i] = 1.0
    rotT = np.ascontiguousarray(R.T)

    j = np.arange(4)[:, None, None]
    r = np.arange(128)[None, :, None]
    q = np.arange(512)[None, None, :]
    mask01 = np.ascontiguousarray(
        ((128 * j + r) <= q).astype(BF16NP).transpose(1, 0, 2))  # [128,4,512]

    hsT = hs.T

    in_maps = []
    for core in range(N_CORES):
        own = np.concatenate(
            [np.arange(512 * r + 64 * core, 512 * r + 64 * core + 64)
             for r in range(4)])
        hT_own = hsT[:, own]
        hTh_np, hTl_np = _split8(hT_own)

        def _h_prep(h8):                                    # [128, DP, 2, 256]
            t = h8.reshape(c.DP, 2, 128, c.LOC).transpose(2, 0, 1, 3)
            return np.ascontiguousarray(t)

        hsel = np.arange(core * c.HPC, (core + 1) * c.HPC)
        wkbv = np.empty((c.KVCH, 128, 2, 512), np.float32)
        wkbv[:, :, 0, :] = wkv_all[:, hsel, :nope].reshape(c.KVCH, 128, 512)
        wkbv[:, :, 1, :] = wkv_all[:, hsel, nope:].reshape(c.KVCH, 128, 512)
        wkbv = np.ascontiguousarray(
            wkbv.transpose(1, 0, 2, 3)).astype(BF16NP)      # [128,KVCH,2,512]

        wo_r = Wo.reshape(c.H, vd, c.D)[hsel] * SWO
        wo8 = wo_r.reshape(2, 2, 128, 32, 128).transpose(2, 3, 0, 1, 4)
        woh_np, wol_np = _split8(np.ascontiguousarray(wo8))  # [128,32,2,2,128]

        in_maps.append({
            "hTh": _h_prep(hTh_np), "hTl": _h_prep(hTl_np),
            "wa_hl": wa_hl, "wqb_hl": wqb_hl,
            "wkbv": wkbv, "woh": woh_np, "wol": wol_np,
            "cosq": cosT[:, own].astype(BF16NP),
            "sinq": sinT[:, own].astype(BF16NP),
            "cosk": (cosT[:64, own] / SA).astype(BF16NP),
            "sink": (sinT[:64, own] / SA).astype(BF16NP),
            "rotT": rotT,
            "ones_f": np.ones((128, 128), np.float32),
            "mask01": mask01,
        })
    return in_maps


# --------------------------------------------------------------------------
# kernel builder
# --------------------------------------------------------------------------

def build(cfg):
    c = cfg
    nc = bacc.Bacc("TRN2", target_bir_lowering=False, debug=False,
                   num_devices=N_CORES)

    hTh_d = nc.declare_dram_parameter("hTh", [128, c.DP, 2, c.LOC], FP8, isOutput=False)
    hTl_d = nc.declare_dram_parameter("hTl", [128, c.DP, 2, c.LOC], FP8, isOutput=False)
    wa_d = nc.declare_dram_parameter("wa_hl", [c.AM, 128, 2, c.DP, 2, 128], FP8, isOutput=False)
    wqb_d = nc.declare_dram_parameter("wqb_hl", [12, 128, 4, 2, c.QP, 2, 128], FP8, isOutput=False)
    wkbv_d = nc.declare_dram_parameter("wkbv", [128, c.KVCH, 2, 512], BF16, isOutput=False)
    woh_d = nc.declare_dram_parameter("woh", [128, 32, 2, 2, 128], FP8, isOutput=False)
    wol_d = nc.declare_dram_parameter("wol", [128, 32, 2, 2, 128], FP8, isOutput=False)
    cosq_d = nc.declare_dram_parameter("cosq", [128, c.LOC], BF16, isOutput=False)
    sinq_d = nc.declare_dram_parameter("sinq", [128, c.LOC], BF16, isOutput=False)
    cosk_d = nc.declare_dram_parameter("cosk", [64, c.LOC], BF16, isOutput=False)
    sink_d = nc.declare_dram_parameter("sink", [64, c.LOC], BF16, isOutput=False)
    rot_d = nc.declare_dram_parameter("rotT", [128, 128], F32R, isOutput=False)
    ones_d = nc.declare_dram_parameter("ones_f", [128, 128], F32R, isOutput=False)
    mask_d = nc.declare_dram_parameter("mask01", [128, 4, 512], BF16, isOutput=False)
    out_d = nc.declare_dram_parameter("outT", [32, 128, c.S], BF16, isOutput=True)

    gin = nc.dram_tensor("ckv_gin", [c.GR, c.LOC], BF16)
    gout = nc.dram_tensor("ckv_gout", [N_CORES, c.GR, c.LOC], BF16,
                          addr_space="Shared")
    a2a_in = nc.dram_tensor("a2a_in", [4, N_CORES, 6, 128, 64], BF16)
    a2a_out = nc.dram_tensor("a2a_out", [4, N_CORES, 6, 128, 64], BF16)

    with tile.TileContext(nc) as tc:
        with tc.tile_pool(name="persist", bufs=1) as pp:
            cosq = pp.tile([128, c.LOC], BF16, name="cosq")
            sinq = pp.tile([128, c.LOC], BF16, name="sinq")
            cosk = pp.tile([64, c.LOC], BF16, name="cosk")
            sink = pp.tile([64, c.LOC], BF16, name="sink")
            rot_sb = pp.tile([128, 128], F32R, name="rot_sb")
            ones_sb = pp.tile([128, 128], F32R, name="ones_sb")
            hTh = pp.tile([128, c.DP, 2, c.LOC], FP8, name="hTh")
            hTl = pp.tile([128, c.DP, 2, c.LOC], FP8, name="hTl")
            nc.sync.dma_start(hTh[:], hTh_d.ap())
            nc.sync.dma_start(hTl[:], hTl_d.ap())
            for t, d in ((ones_sb, ones_d), (rot_sb, rot_d), (cosk, cosk_d),
                         (sink, sink_d), (cosq, cosq_d), (sinq, sinq_d)):
                nc.scalar.dma_start(t[:], d.ap())
            ones_col_f = ones_sb[:, 0:1]
            ones_row_f = ones_sb[0:1, :]
            ones_col_b = pp.tile([128, 1], BF16, name="ones_col_b")
            nc.vector.memset(ones_col_b[:], 1.0)
            woh_sb = pp.tile([128, 32, 2, 2, 128], FP8, name="woh_sb")
            wol_sb = pp.tile([128, 32, 2, 2, 128], FP8, name="wol_sb")
            mask_sb = pp.tile([128, 4, 512], BF16, name="mask_sb")

            # ================= phase A: local a-projections ===============
            with tc.tile_pool(name="pA_w", bufs=3) as pAw, \
                 tc.tile_pool(name="pA_ev", bufs=4) as pAe, \
                 tc.tile_pool(name="pA_keep", bufs=1) as pAk, \
                 tc.tile_pool(name="pA_ps", bufs=2, space="PSUM") as psA, \
                 tc.tile_pool(name="pA_aux", bufs=1, space="PSUM") as psX, \
                 tc.tile_pool(name="pA_ps1", bufs=1, space="PSUM") as psA1:

                def aproj(m, w_sb, planes):
                    """fp8 hi/lo a-proj for chunk m -> psum [128, LOC]."""
                    ps = psA.tile([128, c.LOC], F32, name="psA")
                    terms = [(0, hTh), (0, hTl)] + ([(1, hTh)] if planes == 2
                                                   else [])
                    i = 0
                    n = len(terms) * c.DP
                    for (pl, h) in terms:
                        for kp in range(c.DP):
                            nc.tensor.matmul(
                                ps[:], w_sb[:, pl, kp, :, :], h[:, kp, :, :],
                                start=(i == 0), stop=(i == n - 1),
                                perf_mode=DR)
                            i += 1
                    return ps

                def load_wa(m):
                    planes = 2
                    w = pAw.tile([128, 2, c.DP, 2, 128], FP8, name="wa_sb")
                    nc.sync.dma_start(
                        w[:, 0:planes].rearrange("p a k i x -> p a (k i x)"),
                        wa_d.ap()[m][:, 0:planes]
                        .rearrange("p a k i x -> p a (k i x)"))
                    return w

                morder = list(range(c.QRCH, c.AM)) + list(range(c.QRCH))
                wa_tiles = {}
                for mm in morder[:2]:
                    wa_tiles[mm] = load_wa(mm)

                def get_wa(idx):
                    m = morder[idx]
                    if idx + 2 < len(morder):
                        wa_tiles[morder[idx + 2]] = load_wa(morder[idx + 2])
                    return wa_tiles.pop(m)

                # ---- ckv chunks (m=12..15) + rope key (m=16) ----
                ssc = psA1.tile([1, c.LOC], F32, name="ss_ps")
                c_ev = []
                for mc in range(c.KVCH):
                    ps = aproj(c.QRCH + mc, get_wa(mc), 2)
                    ev = pAk.tile([128, c.LOC], F32, name=f"c_ev{mc}")
                    nc.scalar.activation(ev[:], ps[:], AF.Copy)
                    c_ev.append(ev)
                    x2 = pAe.tile([128, c.LOC], F32R, name="x2")
                    nc.vector.tensor_mul(x2[:], ev[:], ev[:])
                    nc.tensor.matmul(ssc[:], ones_col_f, x2[:],
                                     start=(mc == 0), stop=(mc == c.KVCH - 1))
                ps = aproj(c.AM - 1, get_wa(c.KVCH), 2)
                kr = pAe.tile([64, c.LOC], F32R, name="kr")
                nc.scalar.activation(kr[:], ps[0:64, :], AF.Copy)
                rps = psX.tile([128, c.LOC], F32, name="aux_ps")
                nc.tensor.matmul(rps[0:64, :], rot_sb[0:64, 0:64], kr[:])
                rk = pAe.tile([64, c.LOC], F32, name="rk")
                nc.vector.tensor_copy(rk[:], rps[0:64, :])
                ra = pAe.tile([64, c.LOC], F32, name="ra")
                rb = pAe.tile([64, c.LOC], F32, name="rb")
                nc.vector.tensor_mul(ra[:], kr[:], cosk[:])
                nc.vector.tensor_mul(rb[:], rk[:], sink[:])
                kro = pAe.tile([64, c.LOC], BF16, name="kro")
                nc.vector.tensor_add(kro[:], ra[:], rb[:])
                nc.gpsimd.dma_start(gin.ap()[c.KVCH * 128:c.GR, :], kro[:])

                t1 = pAe.tile([1, c.LOC], F32, name="rms_t")
                nc.vector.tensor_scalar(
                    t1[:], ssc[:], 1.0 / c.KVR, SA * SA * EPS,
                    mybir.AluOpType.mult, mybir.AluOpType.add)
                st = pAe.tile([1, c.LOC], F32, name="rms_st")
                nc.scalar.activation(st[:], t1[:], AF.Sqrt)
                rsc = pAe.tile([1, c.LOC], F32R, name="rsc")
                with nc.allow_low_precision(reason="f32r for PE broadcast"):
                    nc.vector.reciprocal(rsc[:], st[:])
                bc_ps = psX.tile([128, c.LOC], F32, name="aux_ps")
                nc.tensor.matmul(bc_ps[:], ones_row_f, rsc[:])
                bcc = pAe.tile([128, c.LOC], F32, name="bcc")
                nc.vector.tensor_copy(bcc[:], bc_ps[:])
                cn4 = pAe.tile([128, c.KVCH, c.LOC], BF16, name="cn4")
                for mc in range(c.KVCH):
                    nc.vector.tensor_mul(cn4[:, mc, :], c_ev[mc][:], bcc[:])
                nc.gpsimd.dma_start(
                    gin.ap()[0:c.KVCH * 128, :]
                    .rearrange("(k p) x -> p k x", k=c.KVCH), cn4[:])

                # ---- collective 1: AllGather normalized ckv + rope key ----
                nc.gpsimd.collective_compute(
                    "AllGather", mybir.AluOpType.bypass, replica_groups=GRP,
                    ins=[gin.ap()], outs=[gout.ap()])

                # ---- qa chunks (m=0..11): raw hi/lo, rms applied later --
                ssq = psA1.tile([1, c.LOC], F32, name="ss_ps")
                qah = pp.tile([128, c.QP, 2, c.LOC], FP8, name="qah")
                qal = pp.tile([128, c.QP, 2, c.LOC], FP8, name="qal")
                for m in range(c.QRCH):
                    ps = aproj(m, get_wa(c.KVCH + 1 + m), 2)
                    ev = pAe.tile([128, c.LOC], F32, name="qa_ev")
                    nc.scalar.activation(ev[:], ps[:], AF.Copy, scale=1.0 / SA)
                    x2 = pAe.tile([128, c.LOC], F32R, name="x2")
                    nc.vector.tensor_mul(x2[:], ev[:], ev[:])
                    nc.tensor.matmul(ssq[:], ones_col_f, x2[:],
                                     start=(m == 0), stop=(m == c.QRCH - 1))
                    dst_h = qah[:, m // 2, m % 2, :]
                    nc.vector.tensor_copy(dst_h, ev[:])
                    df = pAe.tile([128, c.LOC], F32, name="df")
                    nc.vector.tensor_sub(df[:], ev[:], dst_h)
                    nc.vector.tensor_copy(qal[:, m // 2, m % 2, :], df[:])
                t2 = pAe.tile([1, c.LOC], F32, name="rms_t2")
                nc.vector.tensor_scalar(
                    t2[:], ssq[:], 1.0 / c.QR, EPS,
                    mybir.AluOpType.mult, mybir.AluOpType.add)
                st2 = pAe.tile([1, c.LOC], F32, name="rms_st2")
                nc.scalar.activation(st2[:], t2[:], AF.Sqrt)
                rsq = pAe.tile([1, c.LOC], F32R, name="rsq")
                with nc.allow_low_precision(reason="f32r for PE broadcast"):
                    nc.vector.reciprocal(rsq[:], st2[:])
                bq_ps = psX.tile([128, c.LOC], F32, name="aux_ps")
                nc.tensor.matmul(bq_ps[:], ones_row_f, rsq[:])
                bcq = pp.tile([128, c.LOC], F32, name="bcq")
                nc.vector.tensor_copy(bcq[:], bq_ps[:])
                cosqn = pp.tile([128, c.LOC], F32, name="cosqn")
                sinqn = pp.tile([128, c.LOC], F32, name="sinqn")
                nc.vector.tensor_mul(cosqn[:], cosq[:], bcq[:])
                nc.vector.tensor_mul(sinqn[:], sinq[:], bcq[:])

            # ================= phase Q: local q b-proj (all heads) ========
            with tc.tile_pool(name="pQ_w", bufs=5) as pQw, \
                 tc.tile_pool(name="pQ_ev", bufs=3) as pQe, \
                 tc.tile_pool(name="pQ_ps", bufs=3, space="PSUM") as psQ, \
                 tc.tile_pool(name="pQ_rot", bufs=1, space="PSUM") as psQr:

                def load_wqb(g, eng=None):
                    w = pQw.tile([128, 4, 2, c.QP, 2, 128], FP8, name="wqb_sb")
                    (eng or nc.sync).dma_start(
                        w[:].rearrange("p o a k i x -> p o (a k i x)"),
                        wqb_d.ap()[g]
                        .rearrange("p o a k i x -> p o (a k i x)"))
                    return w

                wq_tiles = {}
                for gg in range(4):
                    wq_tiles[gg] = load_wqb(gg, eng=nc.gpsimd)

                for g in range(12):
                    if g + 4 < 12:
                        wq_tiles[g + 4] = load_wqb(g + 4)
                    wq = wq_tiles.pop(g)
                    qn4 = None
                    qr2 = None
                    if g < 8:
                        qn4 = pQe.tile([128, 4, c.LOC], BF16, name="qn4")
                    else:
                        qr2 = [pQe.tile([128, 2, c.LOC], BF16,
                                        name=f"qr2_{x}") for x in range(2)]
                    for oo in range(4):
                        o = 4 * g + oo
                        ps = psQ.tile([128, c.LOC], F32, name="psQ")
                        i = 0
                        for (pl, q) in ((0, qah), (0, qal), (1, qah)):
                            for kp in range(c.QP):
                                nc.tensor.matmul(
                                    ps[:], wq[:, oo, pl, kp, :, :],
                                    q[:, kp, :, :],
                                    start=(i == 0), stop=(i == 3 * c.QP - 1),
                                    perf_mode=DR)
                                i += 1
                        if o < c.H:
                            qt = pQe.tile([128, c.LOC], F32, name="qt")
                            nc.scalar.activation(
                                qt[:], ps[:], AF.Copy, scale=1.0 / SQ)
                            nc.vector.tensor_mul(qn4[:, oo, :], qt[:], bcq[:])
                        else:
                            j = o - c.H
                            ro = pQe.tile([128, c.LOC], F32R, name="ro")
                            nc.scalar.activation(
                                ro[:], ps[:], AF.Copy, scale=1.0 / SQ)
                            rps = psQr.tile([128, c.LOC], F32, name="rpsQ")
                            nc.tensor.matmul(rps[:], rot_sb[:], ro[:])
                            rk = pQe.tile([128, c.LOC], F32, name="qrk")
                            nc.vector.tensor_copy(rk[:], rps[:])
                            a = pQe.tile([128, c.LOC], F32, name="qra")
                            b = pQe.tile([128, c.LOC], F32, name="qrb")
                            nc.vector.tensor_mul(a[:], ro[:], cosqn[:])
                            nc.vector.tensor_mul(b[:], rk[:], sinqn[:])
                            nc.vector.tensor_add(
                                qr2[oo // 2][:, oo % 2, :], a[:], b[:])
                    if g < 8:
                        for oo in range(4):
                            nc.scalar.dma_start(
                                a2a_in.ap()[:, g, oo]
                                .rearrange("r p x -> p r x"),
                                qn4[:, oo, :].rearrange("p (r x) -> p r x",
                                                        r=4))
                    else:
                        for x in range(2):
                            d = (g - 8) * 2 + x
                            for s in range(2):
                                nc.scalar.dma_start(
                                    a2a_in.ap()[:, d, 4 + s]
                                    .rearrange("r p x -> p r x"),
                                    qr2[x][:, s, :]
                                    .rearrange("p (r x) -> p r x", r=4))

            # ---- collectives 2..5: AllToAll q^T per 512-col round ----
            for r in range(4):
                nc.gpsimd.collective_compute(
                    "AllToAll", mybir.AluOpType.bypass, replica_groups=GRP,
                    ins=[a2a_in.ap()[r]], outs=[a2a_out.ap()[r]])

            # ================= shared B/C residents =======================
            pBC_cm = tc.tile_pool(name="pBC", bufs=1)
            pBC = pBC_cm.__enter__()
            knopeT = [pBC.tile([128, c.S], BF16, name=f"knopeT_{m}")
                      for m in range(c.HPC)]
            v_sb = [pBC.tile([128, c.HPC * c.VD], BF16, name=f"v_sb_{ki}")
                    for ki in range(c.NKI)]
            krope2 = [pBC.tile([128, c.S], BF16, name=f"krope2_{par}")
                      for par in range(2)]
            nc.vector.memset(krope2[0][:], 0.0)
            nc.vector.memset(krope2[1][:], 0.0)
            for par in range(2):
                nc.gpsimd.dma_start(
                    krope2[par][64 * par:64 * par + 64, :]
                    .rearrange("p (g s x) -> p g s x", g=4, s=N_CORES),
                    gout.ap()[:, c.KVCH * 128:c.GR, :]
                    .rearrange("s p (g x) -> p g s x", g=4))

            # ================= phase B: kv b-projection ===================
            with tc.tile_pool(name="pB", bufs=1) as pB, \
                 tc.tile_pool(name="pB_ps", bufs=3, space="PSUM") as psB:
                c_T = []
                for kc in range(c.KVCH):
                    t = pB.tile([128, c.S], BF16, name=f"c_T_{kc}")
                    nc.gpsimd.dma_start(
                        t[:].rearrange("p (g s x) -> p g s x",
                                       g=4, s=N_CORES),
                        gout.ap()[:, kc * 128:(kc + 1) * 128, :]
                        .rearrange("s p (g x) -> p g s x", g=4))
                    c_T.append(t)
                wkbv_sb = pB.tile([128, c.KVCH, 2, 512], BF16, name="wkbv_sb")
                nc.gpsimd.dma_start(
                    wkbv_sb[:].rearrange("p k a x -> p (k a x)"),
                    wkbv_d.ap().rearrange("p k a x -> p (k a x)"))
                nc.sync.dma_start(
                    woh_sb[:].rearrange("p m a b x -> p (m a b x)"),
                    woh_d.ap().rearrange("p m a b x -> p (m a b x)"))
                nc.sync.dma_start(
                    wol_sb[:].rearrange("p m a b x -> p (m a b x)"),
                    wol_d.ap().rearrange("p m a b x -> p (m a b x)"))
                nc.sync.dma_start(
                    mask_sb[:].rearrange("p j x -> p (j x)"),
                    mask_d.ap().rearrange("p j x -> p (j x)"))
                for m in range(c.HPC):
                    for n in range(c.S // 512):
                        ps = psB.tile([128, 512], F32, name="psB")
                        for kc in range(c.KVCH):
                            nc.tensor.matmul(
                                ps[:],
                                wkbv_sb[:, kc, 0, m * 128:(m + 1) * 128],
                                c_T[kc][:, n * 512:(n + 1) * 512],
                                start=(kc == 0), stop=(kc == c.KVCH - 1))
                        nc.scalar.activation(
                            knopeT[m][:, n * 512:(n + 1) * 512], ps[:],
                            AF.Copy)
                for ki in range(c.NKI):
                    ps = psB.tile([128, c.HPC * c.VD], F32, name="psB")
                    for kc in range(c.KVCH):
                        nc.tensor.matmul(
                            ps[:], c_T[kc][:, ki * 128:(ki + 1) * 128],
                            wkbv_sb[:, kc, 1, :], start=(kc == 0),
                            stop=(kc == c.KVCH - 1))
                    nc.scalar.activation(v_sb[ki][:], ps[:], AF.Copy)

            # ================= phase C: attention + out-proj ==============
            with tc.tile_pool(name="pC2", bufs=2) as pC2, \
                 tc.tile_pool(name="pCe", bufs=3) as pCe, \
                 tc.tile_pool(name="pCx", bufs=6) as pCx, \
                 tc.tile_pool(name="pC_mm", bufs=2, space="PSUM") as psM, \
                 tc.tile_pool(name="pC_sT", bufs=3, space="PSUM") as psT, \
                 tc.tile_pool(name="pC_oT", bufs=2, space="PSUM") as psO, \
                 tc.tile_pool(name="pC_den", bufs=1, space="PSUM") as psD:
                for qi in range(c.NQT):
                    q0 = qi * 512
                    qnopeT = []
                    for h in range(c.HPC):
                        t = pC2.tile([128, 512], BF16, name=f"qnopeT_{h}")
                        nc.sync.dma_start(
                            t[:].rearrange("p (s x) -> p s x", s=N_CORES),
                            a2a_out.ap()[qi, :, h].rearrange("s p x -> p s x"))
                        qnopeT.append(t)
                    qropeT = []
                    for j in range(2):
                        t = pC2.tile([128, 512], BF16, name=f"qropeT_{j}")
                        nc.sync.dma_start(
                            t[:].rearrange("p (s x) -> p s x", s=N_CORES),
                            a2a_out.ap()[qi, :, 4 + j]
                            .rearrange("s p x -> p s x"))
                        qropeT.append(t)

                    oT8 = [[pC2.tile([128, 2, 512], FP8, name=f"o{x}_{pr}")
                            for pr in range(2)] for x in range(2)]
                    nki = 4 * (qi + 1)
                    for h in range(c.HPC):
                        oT_ps = psO.tile([128, 512], F32, name="psO")
                        den_ps = psD.tile([1, 512], F32, name="psD")
                        for ki in range(nki):
                            sT_ps = psT.tile([128, 512], F32, name="psT")
                            nc.tensor.matmul(
                                sT_ps[:],
                                knopeT[h][:, ki * 128:(ki + 1) * 128],
                                qnopeT[h][:], start=True, stop=False)
                            nc.tensor.matmul(
                                sT_ps[:],
                                krope2[h % 2][:, ki * 128:(ki + 1) * 128],
                                qropeT[h // 2][:], start=False, stop=True)
                            ex = pCx.tile([128, 512], BF16, name="expT")
                            nc.scalar.activation(ex[:], sT_ps[:], AF.Exp)
                            jj = ki - (nki - 4)
                            if jj >= 0:
                                nc.vector.tensor_mul(ex[:], ex[:],
                                                     mask_sb[:, jj, :])
                            nc.tensor.matmul(den_ps[:], ones_col_b[:], ex[:],
                                             start=(ki == 0),
                                             stop=(ki == nki - 1))
                            nc.tensor.matmul(
                                oT_ps[:],
                                v_sb[ki][:, h * c.VD:(h + 1) * c.VD],
                                ex[:], start=(ki == 0), stop=(ki == nki - 1))
                        rec = pCe.tile([1, 512], F32R, name="rec")
                        with nc.allow_low_precision(reason="f32r broadcast"):
                            nc.vector.reciprocal(rec[:], den_ps[:])
                        bc_ps = psM.tile([128, 512], F32, name="psm")
                        nc.tensor.matmul(bc_ps[:], ones_row_f, rec[:])
                        bc_sb = pCe.tile([128, 512], F32, name="bc_sb")
                        nc.vector.tensor_copy(bc_sb[:], bc_ps[:])
                        ov = pCe.tile([128, 512], F32, name="ov")
                        nc.vector.tensor_mul(ov[:], oT_ps[:], bc_sb[:])
                        dst_h = oT8[0][h // 2][:, h % 2, :]
                        nc.vector.tensor_copy(dst_h, ov[:])
                        df = pCe.tile([128, 512], F32, name="odf")
                        nc.vector.tensor_sub(df[:], ov[:], dst_h)
                        nc.vector.tensor_copy(oT8[1][h // 2][:, h % 2, :],
                                              df[:])

                    for m4 in range(8):
                        ob4 = pCe.tile([128, 4, 512], BF16, name="ob4")
                        for mm in range(4):
                            m = 4 * m4 + mm
                            ps = psM.tile([128, 512], F32, name="psm")
                            i = 0
                            for pr in range(2):
                                for (w, o8) in ((woh_sb, oT8[0][pr]),
                                                (woh_sb, oT8[1][pr]),
                                                (wol_sb, oT8[0][pr])):
                                    nc.tensor.matmul(
                                        ps[:], w[:, m, pr, :, :], o8[:],
                                        start=(i == 0), stop=(i == 5),
                                        perf_mode=DR)
                                    i += 1
                            nc.scalar.activation(
                                ob4[:, mm, :], ps[:], AF.Copy,
                                scale=1.0 / SWO)
                        nc.scalar.dma_start(
                            out_d.ap()[4 * m4:4 * m4 + 4, :, q0:q0 + 512]
                            .rearrange("m p x -> p m x"), ob4[:])
            pBC_cm.__exit__(None, None, None)
    nc.compile()
    return nc


# --------------------------------------------------------------------------
# public entry point
# --------------------------------------------------------------------------

_CACHED = {}


def _get_nc(cfg):
    if cfg not in _CACHED:
        _CACHED[cfg] = build(cfg)
    return _CACHED[cfg]


def kernel(hidden_states, Wq_a, q_a_ln_w, Wq_b, Wkv_a, kv_a_ln_w, Wkv_b, Wo):
    cfg = FULL
    in_maps = prep_inputs(cfg, hidden_states, Wq_a, q_a_ln_w, Wq_b, Wkv_a,
                          kv_a_ln_w, Wkv_b, Wo)
    nc = _get_nc(cfg)
    res = run_bass_kernel_spmd(nc, in_maps, core_ids=list(range(N_CORES)))
    acc = np.zeros((32, 128, cfg.S), np.float32)
    for r in res.results:
        acc += np.asarray(r["outT"], np.float32)
    out = acc.reshape(cfg.D, cfg.S).T
    return np.ascontiguousarray(out).reshape(1, cfg.S, cfg.D)



# revision 6
# speedup vs baseline: 1.0494x; 1.0494x over previous
"""DeepSeek-V3 MLA forward (B=1, S=2048, D=4096, H=32) on 8 TRN2 NeuronCores.

v2: sequence-sharded low-rank a-projections + in-kernel collectives.

Structure (per core c of 8):
  * Core c owns 256 seq columns: 64-col blocks {512r + 64c : r in 0..3}.
  * Phase A (local): a-projections computed only for the owned columns in
    fp8e4 DoubleRow 3-term hi/lo (error ~1e-3, 0.5 cyc/row). RMS scales
    applied locally; weights pre-scaled by 64/512 into fp8 range with the
    inverse folded into rms (scale-invariant), rope tables, or evacuation
    scales.
  * Normalized ckv + rope key AllGather'ed (2.4MB, ~75us, overlapped).
  * Phase Q (local): q b-projection for ALL 32 heads on owned columns,
    fp8 DoubleRow 3-term, weights streamed from DRAM; rope applied locally.
  * 4 chunked AllToAlls redistribute q^T feature-major to head-owners
    (core d owns heads 4d..4d+3), one per 512-col query tile, pipelined
    against attention.
  * Phase B: kv b-projection from gathered ckv (bf16).
  * Phase C: causal attention with transposed scores + fp8 DoubleRow
    3-term out-projection; host sums the 8 partial out-projections.

Engine budget: PE ~360us busy; evacuations spread DVE/ACT; DMAs batched and
issued from producer engines to avoid SP sequencer head-of-line blocking.
"""

from dataclasses import dataclass

import ml_dtypes
import numpy as np

import concourse.bass as bass
import concourse.mybir as mybir
import concourse.tile as tile
from concourse import bacc
from concourse.bass_utils import run_bass_kernel_spmd

F32 = mybir.dt.float32
F32R = mybir.dt.float32r
BF16 = mybir.dt.bfloat16
FP8 = mybir.dt.float8e4
AF = mybir.ActivationFunctionType
DR = mybir.MatmulPerfMode.DoubleRow
BF16NP = ml_dtypes.bfloat16
E4M3 = ml_dtypes.float8_e4m3

N_CORES = 8
GRP = [[0, 1, 2, 3, 4, 5, 6, 7]]
EPS = 1e-6
THETA = 10000.0

SA = 64.0     # a-proj weight prescale (folded out via rms / rope tables)
SQ = 512.0    # q b-proj weight prescale (folded out at evacuation)
SWO = 64.0    # out-proj weight prescale (folded out at evacuation)


@dataclass(frozen=True)
class Cfg:
    S: int = 2048
    D: int = 4096
    QR: int = 1536
    KVR: int = 512
    H: int = 32
    HPC: int = 4
    NOPE: int = 128
    ROPE: int = 64
    VD: int = 128

    @property
    def DP(self):          # 128x2 contraction pairs in D
        return self.D // 256

    @property
    def QP(self):          # pairs in q lora rank
        return self.QR // 256

    @property
    def QRCH(self):
        return self.QR // 128

    @property
    def KVCH(self):
        return self.KVR // 128

    @property
    def AM(self):          # a-proj out chunks: 12 qa + 4 ckv + 1 rope(pad)
        return self.QRCH + self.KVCH + 1

    @property
    def LOC(self):         # owned columns per core
        return self.S // N_CORES

    @property
    def NQT(self):
        return self.S // 512

    @property
    def NKI(self):
        return self.S // 128

    @property
    def QCH(self):         # q b-proj out chunks: 32 nope + 16 rope
        return self.H + self.H * self.ROPE // 128

    @property
    def GR(self):          # gathered rows: 4*128 ckv + 64 rope key
        return self.KVCH * 128 + self.ROPE


FULL = Cfg()


# --------------------------------------------------------------------------
# host-side input preparation
# --------------------------------------------------------------------------

def _rope_perm(rope):
    return np.concatenate([np.arange(0, rope, 2), np.arange(1, rope, 2)])


def _split8(x):
    hi = x.astype(E4M3)
    lo = (x - hi.astype(np.float32)).astype(E4M3)
    return hi, lo


def prep_inputs(cfg, hidden_states, Wq_a, q_a_ln_w, Wq_b, Wkv_a, kv_a_ln_w,
                Wkv_b, Wo):
    c = cfg
    hs = np.asarray(hidden_states, np.float32).reshape(c.S, c.D)
    Wq_a = np.asarray(Wq_a, np.float32)
    Wq_b = np.asarray(Wq_b, np.float32)
    Wkv_a = np.asarray(Wkv_a, np.float32)
    Wkv_b = np.asarray(Wkv_b, np.float32)
    Wo = np.asarray(Wo, np.float32)
    q_a_ln_w = np.asarray(q_a_ln_w, np.float32)
    kv_a_ln_w = np.asarray(kv_a_ln_w, np.float32)

    nope, rope, vd = c.NOPE, c.ROPE, c.VD
    qd = nope + rope

    # combined a-proj weight (x SA), rope cols permuted, padded to 17*128;
    # hi/lo planes packed per m-chunk: [AM, 128, 2, DP, 2, 128]
    perm_a = _rope_perm(rope)
    Wkv_a_p = np.concatenate(
        [Wkv_a[:, :c.KVR], Wkv_a[:, c.KVR:][:, perm_a]], axis=1)
    wa = np.concatenate([Wq_a, Wkv_a_p], axis=1) * SA      # [D, 2112]
    wa = np.pad(wa, ((0, 0), (0, c.AM * 128 - wa.shape[1])))
    wah_np, wal_np = _split8(wa)

    def _wa_prep(w8):                                      # [AM,128,DP,2,128]
        t = w8.reshape(c.DP, 2, 128, c.AM, 128).transpose(3, 2, 0, 1, 4)
        return np.ascontiguousarray(t)
    wa_hl = np.ascontiguousarray(np.stack(
        [_wa_prep(wah_np), _wa_prep(wal_np)], axis=2))
    # -> [AM, 128, 2, DP, 2, 128]

    # q b-proj weights (x SQ), ln + 1/sqrt(qd) + rope perm folded;
    # out-chunk order: 0..31 = nope of head o; 32+j = rope of heads 2j,2j+1;
    # grouped 4 chunks per DMA: [12, 128, 4, 2, QP, 2, 128]
    scale = qd ** (-0.5)
    wqb_all = (Wq_b * q_a_ln_w[:, None]).reshape(c.QR, c.H, qd) * scale * SQ
    perm = _rope_perm(rope)
    wqb_nope = wqb_all[:, :, :nope]
    wqb_rope = wqb_all[:, :, nope:][:, :, perm]
    cols = [wqb_nope[:, h, :] for h in range(c.H)]
    for j in range(c.H // 2):
        cols.append(np.concatenate(
            [wqb_rope[:, 2 * j, :], wqb_rope[:, 2 * j + 1, :]], axis=1))
    wqb = np.stack(cols, axis=0)                            # [48, QR, 128]
    wqbh_np, wqbl_np = _split8(wqb)

    def _wqb_prep(w8):                                      # [48,128,QP,2,128]
        t = w8.reshape(c.QCH, c.QP, 2, 128, 128).transpose(0, 3, 1, 2, 4)
        return np.ascontiguousarray(t)
    wqb_hl = np.stack([_wqb_prep(wqbh_np), _wqb_prep(wqbl_np)], axis=2)
    # [48, 128, 2, QP, 2, 128] -> [12, 128, 4, 2, QP, 2, 128]
    wqb_hl = np.ascontiguousarray(
        wqb_hl.reshape(12, 4, 128, 2, c.QP, 2, 128).transpose(
            0, 2, 1, 3, 4, 5, 6))

    # kv b-proj weights (bf16), ln folded; packed [128, KVCH, 2, 512]
    wkv_all = (Wkv_b * kv_a_ln_w[:, None]).reshape(c.KVR, c.H, nope + vd)

    # rope tables, feature-major [128, S]
    inv_freq = 1.0 / (THETA ** (np.arange(0, rope, 2, np.float32) / rope))
    freqs = np.outer(np.arange(c.S, dtype=np.float32), inv_freq)
    cosT = np.tile(np.cos(freqs).T, (4, 1)).astype(np.float32)
    sinT = np.tile(np.sin(freqs).T, (4, 1)).astype(np.float32)

    R = np.zeros((128, 128), np.float32)
    for blk in (0, 64):
        for i in range(32):
            R[blk + i, blk + i + 32] = -1.0
            R[blk + i + 32, blk + i] = 1.0
    rotT = np.ascontiguousarray(R.T)

    j = np.arange(4)[:, None, None]
    r = np.arange(128)[None, :, None]
    q = np.arange(512)[None, None, :]
    mask01 = np.ascontiguousarray(
        ((128 * j + r) <= q).astype(BF16NP).transpose(1, 0, 2))  # [128,4,512]

    hsT = hs.T

    in_maps = []
    for core in range(N_CORES):
        own = np.concatenate(
            [np.arange(512 * r + 64 * core, 512 * r + 64 * core + 64)
             for r in range(4)])
        hT_own = hsT[:, own]
        hTh_np, hTl_np = _split8(hT_own)

        def _h_prep(h8):                                    # [128, DP, 2, 256]
            t = h8.reshape(c.DP, 2, 128, c.LOC).transpose(2, 0, 1, 3)
            return np.ascontiguousarray(t)

        hsel = np.arange(core * c.HPC, (core + 1) * c.HPC)
        wkbv = np.empty((c.KVCH, 128, 2, 512), np.float32)
        wkbv[:, :, 0, :] = wkv_all[:, hsel, :nope].reshape(c.KVCH, 128, 512)
        wkbv[:, :, 1, :] = wkv_all[:, hsel, nope:].reshape(c.KVCH, 128, 512)
        wkbv = np.ascontiguousarray(
            wkbv.transpose(1, 0, 2, 3)).astype(BF16NP)      # [128,KVCH,2,512]

        wo_r = Wo.reshape(c.H, vd, c.D)[hsel] * SWO
        wo8 = wo_r.reshape(2, 2, 128, 32, 128).transpose(2, 3, 0, 1, 4)
        woh_np, wol_np = _split8(np.ascontiguousarray(wo8))  # [128,32,2,2,128]

        in_maps.append({
            "hTh": _h_prep(hTh_np), "hTl": _h_prep(hTl_np),
            "wa_hl": wa_hl, "wqb_hl": wqb_hl,
            "wkbv": wkbv, "woh": woh_np, "wol": wol_np,
            "cosq": cosT[:, own].astype(BF16NP),
            "sinq": sinT[:, own].astype(BF16NP),
            "cosk": (cosT[:64, own] / SA).astype(BF16NP),
            "sink": (sinT[:64, own] / SA).astype(BF16NP),
            "rotT": rotT,
            "ones_f": np.ones((128, 128), np.float32),
            "mask01": mask01,
        })
    return in_maps


# --------------------------------------------------------------------------
# kernel builder
# --------------------------------------------------------------------------

def build(cfg):
    c = cfg
    nc = bacc.Bacc("TRN2", target_bir_lowering=False, debug=False,
                   num_devices=N_CORES)

    hTh_d = nc.declare_dram_parameter("hTh", [128, c.DP, 2, c.LOC], FP8, isOutput=False)
    hTl_d = nc.declare_dram_parameter("hTl", [128, c.DP, 2, c.LOC], FP8, isOutput=False)
    wa_d = nc.declare_dram_parameter("wa_hl", [c.AM, 128, 2, c.DP, 2, 128], FP8, isOutput=False)
    wqb_d = nc.declare_dram_parameter("wqb_hl", [12, 128, 4, 2, c.QP, 2, 128], FP8, isOutput=False)
    wkbv_d = nc.declare_dram_parameter("wkbv", [128, c.KVCH, 2, 512], BF16, isOutput=False)
    woh_d = nc.declare_dram_parameter("woh", [128, 32, 2, 2, 128], FP8, isOutput=False)
    wol_d = nc.declare_dram_parameter("wol", [128, 32, 2, 2, 128], FP8, isOutput=False)
    cosq_d = nc.declare_dram_parameter("cosq", [128, c.LOC], BF16, isOutput=False)
    sinq_d = nc.declare_dram_parameter("sinq", [128, c.LOC], BF16, isOutput=False)
    cosk_d = nc.declare_dram_parameter("cosk", [64, c.LOC], BF16, isOutput=False)
    sink_d = nc.declare_dram_parameter("sink", [64, c.LOC], BF16, isOutput=False)
    rot_d = nc.declare_dram_parameter("rotT", [128, 128], F32R, isOutput=False)
    ones_d = nc.declare_dram_parameter("ones_f", [128, 128], F32R, isOutput=False)
    mask_d = nc.declare_dram_parameter("mask01", [128, 4, 512], BF16, isOutput=False)
    out_d = nc.declare_dram_parameter("outT", [32, 128, c.S], BF16, isOutput=True)

    gin = nc.dram_tensor("ckv_gin", [c.GR, c.LOC], BF16)
    gout = nc.dram_tensor("ckv_gout", [N_CORES, c.GR, c.LOC], BF16,
                          addr_space="Shared")
    a2a_in = nc.dram_tensor("a2a_in", [4, N_CORES, 6, 128, 64], BF16)
    a2a_out = nc.dram_tensor("a2a_out", [4, N_CORES, 6, 128, 64], BF16)

    with tile.TileContext(nc) as tc:
        with tc.tile_pool(name="persist", bufs=1) as pp:
            cosq = pp.tile([128, c.LOC], BF16, name="cosq")
            sinq = pp.tile([128, c.LOC], BF16, name="sinq")
            cosk = pp.tile([64, c.LOC], BF16, name="cosk")
            sink = pp.tile([64, c.LOC], BF16, name="sink")
            rot_sb = pp.tile([128, 128], F32R, name="rot_sb")
            ones_sb = pp.tile([128, 128], F32R, name="ones_sb")
            hTh = pp.tile([128, c.DP, 2, c.LOC], FP8, name="hTh")
            hTl = pp.tile([128, c.DP, 2, c.LOC], FP8, name="hTl")
            nc.sync.dma_start(hTh[:], hTh_d.ap())
            nc.sync.dma_start(hTl[:], hTl_d.ap())
            for t, d in ((ones_sb, ones_d), (rot_sb, rot_d), (cosk, cosk_d),
                         (sink, sink_d), (cosq, cosq_d), (sinq, sinq_d)):
                nc.scalar.dma_start(t[:], d.ap())
            ones_col_f = ones_sb[:, 0:1]
            ones_row_f = ones_sb[0:1, :]
            ones_col_b = pp.tile([128, 1], BF16, name="ones_col_b")
            nc.vector.memset(ones_col_b[:], 1.0)
            woh_sb = pp.tile([128, 32, 2, 2, 128], FP8, name="woh_sb")
            wol_sb = pp.tile([128, 32, 2, 2, 128], FP8, name="wol_sb")
            mask_sb = pp.tile([128, 4, 512], BF16, name="mask_sb")

            # ================= phase A: local a-projections ===============
            with tc.tile_pool(name="pA_w", bufs=4) as pAw, \
                 tc.tile_pool(name="pA_ev", bufs=4) as pAe, \
                 tc.tile_pool(name="pA_keep", bufs=1) as pAk, \
                 tc.tile_pool(name="pA_ps", bufs=2, space="PSUM") as psA, \
                 tc.tile_pool(name="pA_aux", bufs=1, space="PSUM") as psX, \
                 tc.tile_pool(name="pA_ps1", bufs=1, space="PSUM") as psA1:

                def aproj(m, w_sb, planes):
                    """fp8 hi/lo a-proj for chunk m -> psum [128, LOC]."""
                    ps = psA.tile([128, c.LOC], F32, name="psA")
                    terms = [(0, hTh), (0, hTl)] + ([(1, hTh)] if planes == 2
                                                   else [])
                    i = 0
                    n = len(terms) * c.DP
                    for (pl, h) in terms:
                        for kp in range(c.DP):
                            nc.tensor.matmul(
                                ps[:], w_sb[:, pl, kp, :, :], h[:, kp, :, :],
                                start=(i == 0), stop=(i == n - 1),
                                perf_mode=DR)
                            i += 1
                    return ps

                def load_wa(m):
                    planes = 2
                    w = pAw.tile([128, 2, c.DP, 2, 128], FP8, name="wa_sb")
                    nc.sync.dma_start(
                        w[:, 0:planes].rearrange("p a k i x -> p a (k i x)"),
                        wa_d.ap()[m][:, 0:planes]
                        .rearrange("p a k i x -> p a (k i x)"))
                    return w

                morder = list(range(c.QRCH, c.AM)) + list(range(c.QRCH))
                wa_tiles = {}
                for mm in morder[:3]:
                    wa_tiles[mm] = load_wa(mm)

                def get_wa(idx):
                    m = morder[idx]
                    if idx + 3 < len(morder):
                        wa_tiles[morder[idx + 3]] = load_wa(morder[idx + 3])
                    return wa_tiles.pop(m)

                # ---- ckv chunks (m=12..15) + rope key (m=16) ----
                ssc = psA1.tile([1, c.LOC], F32, name="ss_ps")
                c_ev = []
                for mc in range(c.KVCH):
                    ps = aproj(c.QRCH + mc, get_wa(mc), 2)
                    ev = pAk.tile([128, c.LOC], F32, name=f"c_ev{mc}")
                    nc.scalar.activation(ev[:], ps[:], AF.Copy)
                    c_ev.append(ev)
                    x2 = pAe.tile([128, c.LOC], F32R, name="x2")
                    nc.vector.tensor_mul(x2[:], ev[:], ev[:])
                    nc.tensor.matmul(ssc[:], ones_col_f, x2[:],
                                     start=(mc == 0), stop=(mc == c.KVCH - 1))
                ps = aproj(c.AM - 1, get_wa(c.KVCH), 2)
                kr = pAe.tile([64, c.LOC], F32R, name="kr")
                nc.scalar.activation(kr[:], ps[0:64, :], AF.Copy)
                rps = psX.tile([128, c.LOC], F32, name="aux_ps")
                nc.tensor.matmul(rps[0:64, :], rot_sb[0:64, 0:64], kr[:])
                rk = pAe.tile([64, c.LOC], F32, name="rk")
                nc.vector.tensor_copy(rk[:], rps[0:64, :])
                ra = pAe.tile([64, c.LOC], F32, name="ra")
                rb = pAe.tile([64, c.LOC], F32, name="rb")
                nc.vector.tensor_mul(ra[:], kr[:], cosk[:])
                nc.vector.tensor_mul(rb[:], rk[:], sink[:])
                kro = pAe.tile([64, c.LOC], BF16, name="kro")
                nc.vector.tensor_add(kro[:], ra[:], rb[:])
                nc.gpsimd.dma_start(gin.ap()[c.KVCH * 128:c.GR, :], kro[:])

                t1 = pAe.tile([1, c.LOC], F32, name="rms_t")
                nc.vector.tensor_scalar(
                    t1[:], ssc[:], 1.0 / c.KVR, SA * SA * EPS,
                    mybir.AluOpType.mult, mybir.AluOpType.add)
                st = pAe.tile([1, c.LOC], F32, name="rms_st")
                nc.scalar.activation(st[:], t1[:], AF.Sqrt)
                rsc = pAe.tile([1, c.LOC], F32R, name="rsc")
                with nc.allow_low_precision(reason="f32r for PE broadcast"):
                    nc.vector.reciprocal(rsc[:], st[:])
                bc_ps = psX.tile([128, c.LOC], F32, name="aux_ps")
                nc.tensor.matmul(bc_ps[:], ones_row_f, rsc[:])
                bcc = pAe.tile([128, c.LOC], F32, name="bcc")
                nc.vector.tensor_copy(bcc[:], bc_ps[:])
                cn4 = pAe.tile([128, c.KVCH, c.LOC], BF16, name="cn4")
                for mc in range(c.KVCH):
                    nc.vector.tensor_mul(cn4[:, mc, :], c_ev[mc][:], bcc[:])
                nc.gpsimd.dma_start(
                    gin.ap()[0:c.KVCH * 128, :]
                    .rearrange("(k p) x -> p k x", k=c.KVCH), cn4[:])

                # ---- collective 1: AllGather normalized ckv + rope key ----
                nc.gpsimd.collective_compute(
                    "AllGather", mybir.AluOpType.bypass, replica_groups=GRP,
                    ins=[gin.ap()], outs=[gout.ap()])

                # ---- qa chunks (m=0..11): raw hi/lo, rms applied later --
                ssq = psA1.tile([1, c.LOC], F32, name="ss_ps")
                qah = pp.tile([128, c.QP, 2, c.LOC], FP8, name="qah")
                qal = pp.tile([128, c.QP, 2, c.LOC], FP8, name="qal")
                for m in range(c.QRCH):
                    ps = aproj(m, get_wa(c.KVCH + 1 + m), 2)
                    ev = pAe.tile([128, c.LOC], F32, name="qa_ev")
                    nc.scalar.activation(ev[:], ps[:], AF.Copy, scale=1.0 / SA)
                    x2 = pAe.tile([128, c.LOC], F32R, name="x2")
                    nc.vector.tensor_mul(x2[:], ev[:], ev[:])
                    nc.tensor.matmul(ssq[:], ones_col_f, x2[:],
                                     start=(m == 0), stop=(m == c.QRCH - 1))
                    dst_h = qah[:, m // 2, m % 2, :]
                    nc.vector.tensor_copy(dst_h, ev[:])
                    df = pAe.tile([128, c.LOC], F32, name="df")
                    nc.vector.tensor_sub(df[:], ev[:], dst_h)
                    nc.vector.tensor_copy(qal[:, m // 2, m % 2, :], df[:])
                t2 = pAe.tile([1, c.LOC], F32, name="rms_t2")
                nc.vector.tensor_scalar(
                    t2[:], ssq[:], 1.0 / c.QR, EPS,
                    mybir.AluOpType.mult, mybir.AluOpType.add)
                st2 = pAe.tile([1, c.LOC], F32, name="rms_st2")
                nc.scalar.activation(st2[:], t2[:], AF.Sqrt)
                rsq = pAe.tile([1, c.LOC], F32R, name="rsq")
                with nc.allow_low_precision(reason="f32r for PE broadcast"):
                    nc.vector.reciprocal(rsq[:], st2[:])
                bq_ps = psX.tile([128, c.LOC], F32, name="aux_ps")
                nc.tensor.matmul(bq_ps[:], ones_row_f, rsq[:])
                bcq = pp.tile([128, c.LOC], F32, name="bcq")
                nc.vector.tensor_copy(bcq[:], bq_ps[:])
                cosqn = pp.tile([128, c.LOC], F32, name="cosqn")
                sinqn = pp.tile([128, c.LOC], F32, name="sinqn")
                nc.vector.tensor_mul(cosqn[:], cosq[:], bcq[:])
                nc.vector.tensor_mul(sinqn[:], sinq[:], bcq[:])

            # ================= phase Q: local q b-proj (all heads) ========
            with tc.tile_pool(name="pQ_w", bufs=5) as pQw, \
                 tc.tile_pool(name="pQ_ev", bufs=3) as pQe, \
                 tc.tile_pool(name="pQ_ps", bufs=3, space="PSUM") as psQ, \
                 tc.tile_pool(name="pQ_rot", bufs=1, space="PSUM") as psQr:

                def load_wqb(g, eng=None):
                    w = pQw.tile([128, 4, 2, c.QP, 2, 128], FP8, name="wqb_sb")
                    (eng or nc.sync).dma_start(
                        w[:].rearrange("p o a k i x -> p o (a k i x)"),
                        wqb_d.ap()[g]
                        .rearrange("p o a k i x -> p o (a k i x)"))
                    return w

                wq_tiles = {}
                for gg in range(4):
                    wq_tiles[gg] = load_wqb(gg, eng=nc.gpsimd)

                for g in range(12):
                    if g + 4 < 12:
                        wq_tiles[g + 4] = load_wqb(g + 4)
                    wq = wq_tiles.pop(g)
                    qn4 = None
                    qr2 = None
                    if g < 8:
                        qn4 = pQe.tile([128, 4, c.LOC], BF16, name="qn4")
                    else:
                        qr2 = [pQe.tile([128, 2, c.LOC], BF16,
                                        name=f"qr2_{x}") for x in range(2)]
                    for oo in range(4):
                        o = 4 * g + oo
                        ps = psQ.tile([128, c.LOC], F32, name="psQ")
                        i = 0
                        for (pl, q) in ((0, qah), (0, qal), (1, qah)):
                            for kp in range(c.QP):
                                nc.tensor.matmul(
                                    ps[:], wq[:, oo, pl, kp, :, :],
                                    q[:, kp, :, :],
                                    start=(i == 0), stop=(i == 3 * c.QP - 1),
                                    perf_mode=DR)
                                i += 1
                        if o < c.H:
                            qt = pQe.tile([128, c.LOC], F32, name="qt")
                            nc.scalar.activation(
                                qt[:], ps[:], AF.Copy, scale=1.0 / SQ)
                            nc.vector.tensor_mul(qn4[:, oo, :], qt[:], bcq[:])
                        else:
                            j = o - c.H
                            ro = pQe.tile([128, c.LOC], F32R, name="ro")
                            nc.scalar.activation(
                                ro[:], ps[:], AF.Copy, scale=1.0 / SQ)
                            rps = psQr.tile([128, c.LOC], F32, name="rpsQ")
                            nc.tensor.matmul(rps[:], rot_sb[:], ro[:])
                            rk = pQe.tile([128, c.LOC], F32, name="qrk")
                            nc.vector.tensor_copy(rk[:], rps[:])
                            a = pQe.tile([128, c.LOC], F32, name="qra")
                            b = pQe.tile([128, c.LOC], F32, name="qrb")
                            nc.vector.tensor_mul(a[:], ro[:], cosqn[:])
                            nc.vector.tensor_mul(b[:], rk[:], sinqn[:])
                            nc.vector.tensor_add(
                                qr2[oo // 2][:, oo % 2, :], a[:], b[:])
                    if g < 8:
                        for oo in range(4):
                            nc.scalar.dma_start(
                                a2a_in.ap()[:, g, oo]
                                .rearrange("r p x -> p r x"),
                                qn4[:, oo, :].rearrange("p (r x) -> p r x",
                                                        r=4))
                    else:
                        for x in range(2):
                            d = (g - 8) * 2 + x
                            for s in range(2):
                                nc.scalar.dma_start(
                                    a2a_in.ap()[:, d, 4 + s]
                                    .rearrange("r p x -> p r x"),
                                    qr2[x][:, s, :]
                                    .rearrange("p (r x) -> p r x", r=4))

            # ================= shared B/C residents =======================
            # gout reads issue BEFORE the A2A issues: they only wait on the
            # AllGather, so phase B's inputs land while phase Q streams.
            pBC_cm = tc.tile_pool(name="pBC", bufs=1)
            pBC = pBC_cm.__enter__()
            pB_cm = tc.tile_pool(name="pB", bufs=1)
            pB = pB_cm.__enter__()
            knopeT = [pBC.tile([128, c.S], BF16, name=f"knopeT_{m}")
                      for m in range(c.HPC)]
            v_sb = [pBC.tile([128, c.HPC * c.VD], BF16, name=f"v_sb_{ki}")
                    for ki in range(c.NKI)]
            krope2 = [pBC.tile([128, c.S], BF16, name=f"krope2_{par}")
                      for par in range(2)]
            nc.vector.memset(krope2[0][:], 0.0)
            nc.vector.memset(krope2[1][:], 0.0)
            for par in range(2):
                nc.gpsimd.dma_start(
                    krope2[par][64 * par:64 * par + 64, :]
                    .rearrange("p (g s x) -> p g s x", g=4, s=N_CORES),
                    gout.ap()[:, c.KVCH * 128:c.GR, :]
                    .rearrange("s p (g x) -> p g s x", g=4))
            c_T = []
            for kc in range(c.KVCH):
                t = pB.tile([128, c.S], BF16, name=f"c_T_{kc}")
                nc.gpsimd.dma_start(
                    t[:].rearrange("p (g s x) -> p g s x",
                                   g=4, s=N_CORES),
                    gout.ap()[:, kc * 128:(kc + 1) * 128, :]
                    .rearrange("s p (g x) -> p g s x", g=4))
                c_T.append(t)
            wkbv_sb = pB.tile([128, c.KVCH, 2, 512], BF16, name="wkbv_sb")
            nc.gpsimd.dma_start(
                wkbv_sb[:].rearrange("p k a x -> p (k a x)"),
                wkbv_d.ap().rearrange("p k a x -> p (k a x)"))

            # ---- collectives 2..5: AllToAll q^T per 512-col round ----
            for r in range(4):
                nc.gpsimd.collective_compute(
                    "AllToAll", mybir.AluOpType.bypass, replica_groups=GRP,
                    ins=[a2a_in.ap()[r]], outs=[a2a_out.ap()[r]])

            # ================= phase B: kv b-projection ===================
            with tc.tile_pool(name="pB_ps", bufs=3, space="PSUM") as psB:
                for m in range(c.HPC):
                    for n in range(c.S // 512):
                        ps = psB.tile([128, 512], F32, name="psB")
                        for kc in range(c.KVCH):
                            nc.tensor.matmul(
                                ps[:],
                                wkbv_sb[:, kc, 0, m * 128:(m + 1) * 128],
                                c_T[kc][:, n * 512:(n + 1) * 512],
                                start=(kc == 0), stop=(kc == c.KVCH - 1))
                        nc.scalar.activation(
                            knopeT[m][:, n * 512:(n + 1) * 512], ps[:],
                            AF.Copy)
                for ki in range(c.NKI):
                    ps = psB.tile([128, c.HPC * c.VD], F32, name="psB")
                    for kc in range(c.KVCH):
                        nc.tensor.matmul(
                            ps[:], c_T[kc][:, ki * 128:(ki + 1) * 128],
                            wkbv_sb[:, kc, 1, :], start=(kc == 0),
                            stop=(kc == c.KVCH - 1))
                    nc.scalar.activation(v_sb[ki][:], ps[:], AF.Copy)
            pB_cm.__exit__(None, None, None)

            # ================= phase C: attention + out-proj ==============
            with tc.tile_pool(name="pC2", bufs=2) as pC2, \
                 tc.tile_pool(name="pCe", bufs=3) as pCe, \
                 tc.tile_pool(name="pCx", bufs=6) as pCx, \
                 tc.tile_pool(name="pCacc", bufs=2) as pCa, \
                 tc.tile_pool(name="pC_mm", bufs=2, space="PSUM") as psM, \
                 tc.tile_pool(name="pC_sT", bufs=2, space="PSUM") as psT, \
                 tc.tile_pool(name="pC_oT", bufs=2, space="PSUM") as psO, \
                 tc.tile_pool(name="pC_den", bufs=2, space="PSUM") as psD:
                # out-proj weights + causal mask needed only here; loading
                # now keeps their 13us of DMA out of the A/Q weight stream.
                nc.sync.dma_start(
                    mask_sb[:].rearrange("p j x -> p (j x)"),
                    mask_d.ap().rearrange("p j x -> p (j x)"))
                nc.sync.dma_start(
                    woh_sb[:].rearrange("p m a b x -> p (m a b x)"),
                    woh_d.ap().rearrange("p m a b x -> p (m a b x)"))
                nc.sync.dma_start(
                    wol_sb[:].rearrange("p m a b x -> p (m a b x)"),
                    wol_d.ap().rearrange("p m a b x -> p (m a b x)"))
                for qi in range(c.NQT):
                    q0 = qi * 512
                    qnopeT = []
                    for h in range(c.HPC):
                        t = pC2.tile([128, 512], BF16, name=f"qnopeT_{h}")
                        nc.sync.dma_start(
                            t[:].rearrange("p (s x) -> p s x", s=N_CORES),
                            a2a_out.ap()[qi, :, h].rearrange("s p x -> p s x"))
                        qnopeT.append(t)
                    qropeT = []
                    for j in range(2):
                        t = pC2.tile([128, 512], BF16, name=f"qropeT_{j}")
                        nc.sync.dma_start(
                            t[:].rearrange("p (s x) -> p s x", s=N_CORES),
                            a2a_out.ap()[qi, :, 4 + j]
                            .rearrange("s p x -> p s x"))
                        qropeT.append(t)

                    oT8 = [[pC2.tile([128, 2, 512], FP8, name=f"o{x}_{pr}")
                            for pr in range(2)] for x in range(2)]
                    nki = 4 * (qi + 1)

                    def attn_head(h):
                        """ki loop for head h: softmax numerator into PSUM,
                        denominator accumulated on DVE in bf16."""
                        oT_ps = psO.tile([128, 512], F32, name="psO")
                        acc = pCa.tile([128, 512], BF16, name="den_acc")
                        for ki in range(nki):
                            sT_ps = psT.tile([128, 512], F32, name="psT")
                            nc.tensor.matmul(
                                sT_ps[:],
                                knopeT[h][:, ki * 128:(ki + 1) * 128],
                                qnopeT[h][:], start=True, stop=False)
                            nc.tensor.matmul(
                                sT_ps[:],
                                krope2[h % 2][:, ki * 128:(ki + 1) * 128],
                                qropeT[h // 2][:], start=False, stop=True)
                            ex = pCx.tile([128, 512], BF16, name="expT")
                            nc.scalar.activation(ex[:], sT_ps[:], AF.Exp)
                            jj = ki - (nki - 4)
                            if jj >= 0:
                                nc.vector.tensor_mul(ex[:], ex[:],
                                                     mask_sb[:, jj, :])
                            if ki == 0:
                                nc.vector.tensor_copy(acc[:], ex[:])
                            else:
                                nc.vector.tensor_add(acc[:], acc[:], ex[:])
                            nc.tensor.matmul(
                                oT_ps[:],
                                v_sb[ki][:, h * c.VD:(h + 1) * c.VD],
                                ex[:], start=(ki == 0), stop=(ki == nki - 1))
                        den_ps = psD.tile([1, 512], F32, name="psD")
                        nc.tensor.matmul(den_ps[:], ones_col_b[:], acc[:],
                                         start=True, stop=True)
                        rec = pCe.tile([1, 512], F32R, name="rec")
                        with nc.allow_low_precision(reason="f32r broadcast"):
                            nc.vector.reciprocal(rec[:], den_ps[:])
                        return oT_ps, rec

                    def norm_head(h, oT_ps, rec):
                        """normalize + fp8 hi/lo split of head h's output."""
                        bc_ps = psM.tile([128, 512], F32, name="psm")
                        nc.tensor.matmul(bc_ps[:], ones_row_f, rec[:])
                        bc_sb = pCe.tile([128, 512], F32, name="bc_sb")
                        nc.vector.tensor_copy(bc_sb[:], bc_ps[:])
                        ov = pCe.tile([128, 512], F32, name="ov")
                        nc.vector.tensor_mul(ov[:], oT_ps[:], bc_sb[:])
                        dst_h = oT8[0][h // 2][:, h % 2, :]
                        nc.vector.tensor_copy(dst_h, ov[:])
                        df = pCe.tile([128, 512], F32, name="odf")
                        nc.vector.tensor_sub(df[:], ov[:], dst_h)
                        nc.vector.tensor_copy(oT8[1][h // 2][:, h % 2, :],
                                              df[:])

                    # software-pipeline: head h's attention runs while head
                    # h-1's reciprocal lands, so the broadcast matmul never
                    # stalls PE.
                    prev = None
                    for h in range(c.HPC):
                        cur = attn_head(h)
                        if prev is not None:
                            norm_head(h - 1, *prev)
                        prev = cur
                    norm_head(c.HPC - 1, *prev)

                    for m4 in range(8):
                        ob4 = pCe.tile([128, 4, 512], BF16, name="ob4")
                        for mm in range(4):
                            m = 4 * m4 + mm
                            ps = psM.tile([128, 512], F32, name="psm")
                            i = 0
                            for pr in range(2):
                                for (w, o8) in ((woh_sb, oT8[0][pr]),
                                                (woh_sb, oT8[1][pr]),
                                                (wol_sb, oT8[0][pr])):
                                    nc.tensor.matmul(
                                        ps[:], w[:, m, pr, :, :], o8[:],
                                        start=(i == 0), stop=(i == 5),
                                        perf_mode=DR)
                                    i += 1
                            nc.scalar.activation(
                                ob4[:, mm, :], ps[:], AF.Copy,
                                scale=1.0 / SWO)
                        nc.scalar.dma_start(
                            out_d.ap()[4 * m4:4 * m4 + 4, :, q0:q0 + 512]
                            .rearrange("m p x -> p m x"), ob4[:])
            pBC_cm.__exit__(None, None, None)
    nc.compile()
    return nc


# --------------------------------------------------------------------------
# public entry point
# --------------------------------------------------------------------------

_CACHED = {}


def _get_nc(cfg):
    if cfg not in _CACHED:
        _CACHED[cfg] = build(cfg)
    return _CACHED[cfg]


def kernel(hidden_states, Wq_a, q_a_ln_w, Wq_b, Wkv_a, kv_a_ln_w, Wkv_b, Wo):
    cfg = FULL
    in_maps = prep_inputs(cfg, hidden_states, Wq_a, q_a_ln_w, Wq_b, Wkv_a,
                          kv_a_ln_w, Wkv_b, Wo)
    nc = _get_nc(cfg)
    res = run_bass_kernel_spmd(nc, in_maps, core_ids=list(range(N_CORES)))
    acc = np.zeros((32, 128, cfg.S), np.float32)
    for r in res.results:
        acc += np.asarray(r["outT"], np.float32)
    out = acc.reshape(cfg.D, cfg.S).T
    return np.ascontiguousarray(out).reshape(1, cfg.S, cfg.D)



# revision 16
# speedup vs baseline: 1.1023x; 1.0504x over previous
"""DeepSeek-V3 MLA forward (B=1, S=2048, D=4096, H=32) on 8 TRN2 NeuronCores.

v2: sequence-sharded low-rank a-projections + in-kernel collectives.

Structure (per core c of 8):
  * Core c owns 256 seq columns: 64-col blocks {512r + 64c : r in 0..3}.
  * Phase A (local): a-projections computed only for the owned columns in
    fp8e4 DoubleRow 3-term hi/lo (error ~1e-3, 0.5 cyc/row). RMS scales
    applied locally; weights pre-scaled by 64/512 into fp8 range with the
    inverse folded into rms (scale-invariant), rope tables, or evacuation
    scales.
  * Normalized ckv + rope key AllGather'ed (2.4MB, ~75us, overlapped).
  * Phase Q (local): q b-projection for ALL 32 heads on owned columns,
    fp8 DoubleRow 3-term, weights streamed from DRAM; rope applied locally.
  * 4 chunked AllToAlls redistribute q^T feature-major to head-owners
    (core d owns heads 4d..4d+3), one per 512-col query tile, pipelined
    against attention.
  * Phase B: kv b-projection from gathered ckv (bf16).
  * Phase C: causal attention with transposed scores + fp8 DoubleRow
    3-term out-projection; host sums the 8 partial out-projections.

Engine budget: PE ~360us busy; evacuations spread DVE/ACT; DMAs batched and
issued from producer engines to avoid SP sequencer head-of-line blocking.
"""

from dataclasses import dataclass

import ml_dtypes
import numpy as np

import concourse.bass as bass
import concourse.mybir as mybir
import concourse.tile as tile
from concourse import bacc
from concourse.bass_utils import run_bass_kernel_spmd

F32 = mybir.dt.float32
F32R = mybir.dt.float32r
BF16 = mybir.dt.bfloat16
FP8 = mybir.dt.float8e4
AF = mybir.ActivationFunctionType
DR = mybir.MatmulPerfMode.DoubleRow
BF16NP = ml_dtypes.bfloat16
E4M3 = ml_dtypes.float8_e4m3

N_CORES = 8
GRP = [[0, 1, 2, 3, 4, 5, 6, 7]]
EPS = 1e-6
THETA = 10000.0

SA = 64.0     # a-proj weight prescale (folded out via rms / rope tables)
SQ = 512.0    # q b-proj weight prescale (folded out at evacuation)
SWO = 64.0    # out-proj weight prescale (folded out at evacuation)


@dataclass(frozen=True)
class Cfg:
    S: int = 2048
    D: int = 4096
    QR: int = 1536
    KVR: int = 512
    H: int = 32
    HPC: int = 4
    NOPE: int = 128
    ROPE: int = 64
    VD: int = 128

    @property
    def DP(self):          # 128x2 contraction pairs in D
        return self.D // 256

    @property
    def QP(self):          # pairs in q lora rank
        return self.QR // 256

    @property
    def QRCH(self):
        return self.QR // 128

    @property
    def KVCH(self):
        return self.KVR // 128

    @property
    def AM(self):          # a-proj out chunks: 12 qa + 4 ckv + 1 rope(pad)
        return self.QRCH + self.KVCH + 1

    @property
    def LOC(self):         # owned columns per core
        return self.S // N_CORES

    @property
    def NQT(self):
        return self.S // 512

    @property
    def NKI(self):
        return self.S // 128

    @property
    def QCH(self):         # q b-proj out chunks: 32 nope + 16 rope
        return self.H + self.H * self.ROPE // 128

    @property
    def GR(self):          # gathered rows: 4*128 ckv + 64 rope key
        return self.KVCH * 128 + self.ROPE


FULL = Cfg()


# --------------------------------------------------------------------------
# host-side input preparation
# --------------------------------------------------------------------------

def _rope_perm(rope):
    return np.concatenate([np.arange(0, rope, 2), np.arange(1, rope, 2)])


def _split8(x):
    hi = x.astype(E4M3)
    lo = (x - hi.astype(np.float32)).astype(E4M3)
    return hi, lo


def prep_inputs(cfg, hidden_states, Wq_a, q_a_ln_w, Wq_b, Wkv_a, kv_a_ln_w,
                Wkv_b, Wo):
    c = cfg
    hs = np.asarray(hidden_states, np.float32).reshape(c.S, c.D)
    Wq_a = np.asarray(Wq_a, np.float32)
    Wq_b = np.asarray(Wq_b, np.float32)
    Wkv_a = np.asarray(Wkv_a, np.float32)
    Wkv_b = np.asarray(Wkv_b, np.float32)
    Wo = np.asarray(Wo, np.float32)
    q_a_ln_w = np.asarray(q_a_ln_w, np.float32)
    kv_a_ln_w = np.asarray(kv_a_ln_w, np.float32)

    nope, rope, vd = c.NOPE, c.ROPE, c.VD
    qd = nope + rope

    # combined a-proj weight (x SA), rope cols permuted, padded to 17*128;
    # hi/lo planes packed per m-chunk: [AM, 128, 2, DP, 2, 128]
    perm_a = _rope_perm(rope)
    Wkv_a_p = np.concatenate(
        [Wkv_a[:, :c.KVR], Wkv_a[:, c.KVR:][:, perm_a]], axis=1)
    wa = np.concatenate([Wq_a, Wkv_a_p], axis=1) * SA      # [D, 2112]
    wa = np.pad(wa, ((0, 0), (0, c.AM * 128 - wa.shape[1])))
    wah_np, wal_np = _split8(wa)

    def _wa_prep(w8):                                      # [AM,128,DP,2,128]
        t = w8.reshape(c.DP, 2, 128, c.AM, 128).transpose(3, 2, 0, 1, 4)
        return np.ascontiguousarray(t)
    wa_hl = np.ascontiguousarray(np.stack(
        [_wa_prep(wah_np), _wa_prep(wal_np)], axis=2))
    # -> [AM, 128, 2, DP, 2, 128]

    # q b-proj weights (x SQ), ln + 1/sqrt(qd) + rope perm folded;
    # out-chunk order: 0..31 = nope of head o; 32+j = rope of heads 2j,2j+1;
    # grouped 4 chunks per DMA: [12, 128, 4, 2, QP, 2, 128]
    scale = qd ** (-0.5)
    wqb_all = (Wq_b * q_a_ln_w[:, None]).reshape(c.QR, c.H, qd) * scale * SQ
    perm = _rope_perm(rope)
    wqb_nope = wqb_all[:, :, :nope]
    wqb_rope = wqb_all[:, :, nope:][:, :, perm]
    cols = [wqb_nope[:, h, :] for h in range(c.H)]
    for j in range(c.H // 2):
        cols.append(np.concatenate(
            [wqb_rope[:, 2 * j, :], wqb_rope[:, 2 * j + 1, :]], axis=1))
    wqb = np.stack(cols, axis=0)                            # [48, QR, 128]
    wqbh_np, wqbl_np = _split8(wqb)

    def _wqb_prep(w8):                                      # [48,128,QP,2,128]
        t = w8.reshape(c.QCH, c.QP, 2, 128, 128).transpose(0, 3, 1, 2, 4)
        return np.ascontiguousarray(t)
    wqb_hl = np.stack([_wqb_prep(wqbh_np), _wqb_prep(wqbl_np)], axis=2)
    # [48, 128, 2, QP, 2, 128] -> [12, 128, 4, 2, QP, 2, 128]
    wqb_hl = np.ascontiguousarray(
        wqb_hl.reshape(12, 4, 128, 2, c.QP, 2, 128).transpose(
            0, 2, 1, 3, 4, 5, 6))

    # kv b-proj weights (bf16), ln folded; packed [128, KVCH, 2, 512]
    wkv_all = (Wkv_b * kv_a_ln_w[:, None]).reshape(c.KVR, c.H, nope + vd)

    # rope tables, feature-major [128, S]
    inv_freq = 1.0 / (THETA ** (np.arange(0, rope, 2, np.float32) / rope))
    freqs = np.outer(np.arange(c.S, dtype=np.float32), inv_freq)
    cosT = np.tile(np.cos(freqs).T, (4, 1)).astype(np.float32)
    sinT = np.tile(np.sin(freqs).T, (4, 1)).astype(np.float32)

    R = np.zeros((128, 128), np.float32)
    for blk in (0, 64):
        for i in range(32):
            R[blk + i, blk + i + 32] = -1.0
            R[blk + i + 32, blk + i] = 1.0
    rotT = np.ascontiguousarray(R.T)

    j = np.arange(4)[:, None, None]
    r = np.arange(128)[None, :, None]
    q = np.arange(512)[None, None, :]
    mask01 = np.ascontiguousarray(
        ((128 * j + r) <= q).astype(BF16NP).transpose(1, 0, 2))  # [128,4,512]

    hsT = hs.T

    in_maps = []
    for core in range(N_CORES):
        own = np.concatenate(
            [np.arange(512 * r + 64 * core, 512 * r + 64 * core + 64)
             for r in range(4)])
        hT_own = hsT[:, own]
        hTh_np, hTl_np = _split8(hT_own)

        def _h_prep(h8):                                    # [128, DP, 2, 256]
            t = h8.reshape(c.DP, 2, 128, c.LOC).transpose(2, 0, 1, 3)
            return np.ascontiguousarray(t)

        hsel = np.arange(core * c.HPC, (core + 1) * c.HPC)
        wkbv = np.empty((c.KVCH, 128, 2, 512), np.float32)
        wkbv[:, :, 0, :] = wkv_all[:, hsel, :nope].reshape(c.KVCH, 128, 512)
        wkbv[:, :, 1, :] = wkv_all[:, hsel, nope:].reshape(c.KVCH, 128, 512)
        wkbv = np.ascontiguousarray(
            wkbv.transpose(1, 0, 2, 3)).astype(BF16NP)      # [128,KVCH,2,512]

        wo_r = Wo.reshape(c.H, vd, c.D)[hsel] * SWO
        wo8 = wo_r.reshape(2, 2, 128, 32, 128).transpose(2, 3, 0, 1, 4)
        woh_np, wol_np = _split8(np.ascontiguousarray(wo8))  # [128,32,2,2,128]

        in_maps.append({
            "hTh": _h_prep(hTh_np), "hTl": _h_prep(hTl_np),
            "wa_hl": wa_hl, "wqb_hl": wqb_hl,
            "wkbv": wkbv, "woh": woh_np, "wol": wol_np,
            "cosq": cosT[:, own].astype(BF16NP),
            "sinq": sinT[:, own].astype(BF16NP),
            "cosk": (cosT[:64, own] / SA).astype(BF16NP),
            "sink": (sinT[:64, own] / SA).astype(BF16NP),
            "rotT": rotT,
            "ones_f": np.ones((128, 128), np.float32),
            "mask01": mask01,
        })
    return in_maps


# --------------------------------------------------------------------------
# kernel builder
# --------------------------------------------------------------------------

def build(cfg):
    c = cfg
    nc = bacc.Bacc("TRN2", target_bir_lowering=False, debug=False,
                   num_devices=N_CORES)

    hTh_d = nc.declare_dram_parameter("hTh", [128, c.DP, 2, c.LOC], FP8, isOutput=False)
    hTl_d = nc.declare_dram_parameter("hTl", [128, c.DP, 2, c.LOC], FP8, isOutput=False)
    wa_d = nc.declare_dram_parameter("wa_hl", [c.AM, 128, 2, c.DP, 2, 128], FP8, isOutput=False)
    wqb_d = nc.declare_dram_parameter("wqb_hl", [12, 128, 4, 2, c.QP, 2, 128], FP8, isOutput=False)
    wkbv_d = nc.declare_dram_parameter("wkbv", [128, c.KVCH, 2, 512], BF16, isOutput=False)
    woh_d = nc.declare_dram_parameter("woh", [128, 32, 2, 2, 128], FP8, isOutput=False)
    wol_d = nc.declare_dram_parameter("wol", [128, 32, 2, 2, 128], FP8, isOutput=False)
    cosq_d = nc.declare_dram_parameter("cosq", [128, c.LOC], BF16, isOutput=False)
    sinq_d = nc.declare_dram_parameter("sinq", [128, c.LOC], BF16, isOutput=False)
    cosk_d = nc.declare_dram_parameter("cosk", [64, c.LOC], BF16, isOutput=False)
    sink_d = nc.declare_dram_parameter("sink", [64, c.LOC], BF16, isOutput=False)
    rot_d = nc.declare_dram_parameter("rotT", [128, 128], F32R, isOutput=False)
    ones_d = nc.declare_dram_parameter("ones_f", [128, 128], F32R, isOutput=False)
    mask_d = nc.declare_dram_parameter("mask01", [128, 4, 512], BF16, isOutput=False)
    out_d = nc.declare_dram_parameter("outT", [32, 128, c.S], BF16, isOutput=True)

    gin = nc.dram_tensor("ckv_gin", [c.GR, c.LOC], BF16)
    gout = nc.dram_tensor("ckv_gout", [N_CORES, c.GR, c.LOC], BF16,
                          addr_space="Shared")
    # [round, dest, partition, chunk, col]: per-(r,p) runs of 4 nope chunks
    # are 512B contiguous, and the phase-C read per qi is one 768B-elem DMA.
    a2a_in = nc.dram_tensor("a2a_in", [4, N_CORES, 128, 6, 64], BF16)
    a2a_out = nc.dram_tensor("a2a_out", [4, N_CORES, 128, 6, 64], BF16)

    with tile.TileContext(nc) as tc:
        with tc.tile_pool(name="persist", bufs=1) as pp:
            cosq = pp.tile([128, c.LOC], BF16, name="cosq")
            sinq = pp.tile([128, c.LOC], BF16, name="sinq")
            cosk = pp.tile([64, c.LOC], BF16, name="cosk")
            sink = pp.tile([64, c.LOC], BF16, name="sink")
            rot_sb = pp.tile([128, 128], F32R, name="rot_sb")
            ones_sb = pp.tile([128, 128], F32R, name="ones_sb")
            hTh = pp.tile([128, c.DP, 2, c.LOC], FP8, name="hTh")
            hTl = pp.tile([128, c.DP, 2, c.LOC], FP8, name="hTl")
            nc.sync.dma_start(hTh[:], hTh_d.ap())
            nc.sync.dma_start(hTl[:], hTl_d.ap())
            for t, d in ((ones_sb, ones_d), (rot_sb, rot_d), (cosk, cosk_d),
                         (sink, sink_d), (cosq, cosq_d), (sinq, sinq_d)):
                nc.scalar.dma_start(t[:], d.ap())
            ones_col_f = ones_sb[:, 0:1]
            ones_row_f = ones_sb[0:1, :]
            ones_col_b = pp.tile([128, 1], BF16, name="ones_col_b")
            nc.vector.memset(ones_col_b[:], 1.0)
            woh_sb = pp.tile([128, 32, 2, 2, 128], FP8, name="woh_sb")
            wol_sb = pp.tile([128, 32, 2, 2, 128], FP8, name="wol_sb")
            mask_sb = pp.tile([128, 4, 512], BF16, name="mask_sb")
            # krope2 memset here so the gather-read into it isn't gated on
            # DVE reaching the end of phase Q.
            krope2 = [pp.tile([128, c.S], BF16, name=f"krope2_{par}")
                      for par in range(2)]
            nc.vector.memset(krope2[0][:], 0.0)
            nc.vector.memset(krope2[1][:], 0.0)

            # ================= phase A: local a-projections ===============
            with tc.tile_pool(name="pA_w", bufs=4) as pAw, \
                 tc.tile_pool(name="pA_ev", bufs=4) as pAe, \
                 tc.tile_pool(name="pA_keep", bufs=1) as pAk, \
                 tc.tile_pool(name="pA_ps", bufs=2, space="PSUM") as psA, \
                 tc.tile_pool(name="pA_aux", bufs=1, space="PSUM") as psX, \
                 tc.tile_pool(name="pA_ps1", bufs=1, space="PSUM") as psA1:

                def aproj(m, w_sb, planes):
                    """fp8 hi/lo a-proj for chunk m -> psum [128, LOC]."""
                    ps = psA.tile([128, c.LOC], F32, name="psA")
                    terms = [(0, hTh), (0, hTl)] + ([(1, hTh)] if planes == 2
                                                   else [])
                    i = 0
                    n = len(terms) * c.DP
                    for (pl, h) in terms:
                        for kp in range(c.DP):
                            nc.tensor.matmul(
                                ps[:], w_sb[:, pl, kp, :, :], h[:, kp, :, :],
                                start=(i == 0), stop=(i == n - 1),
                                perf_mode=DR)
                            i += 1
                    return ps

                def load_wa(m):
                    planes = 2
                    w = pAw.tile([128, 2, c.DP, 2, 128], FP8, name="wa_sb")
                    nc.sync.dma_start(
                        w[:, 0:planes].rearrange("p a k i x -> p a (k i x)"),
                        wa_d.ap()[m][:, 0:planes]
                        .rearrange("p a k i x -> p a (k i x)"))
                    return w

                morder = list(range(c.QRCH, c.AM)) + list(range(c.QRCH))
                wa_tiles = {}
                for mm in morder[:3]:
                    wa_tiles[mm] = load_wa(mm)

                def get_wa(idx):
                    m = morder[idx]
                    if idx + 3 < len(morder):
                        wa_tiles[morder[idx + 3]] = load_wa(morder[idx + 3])
                    return wa_tiles.pop(m)

                # ---- ckv chunks (m=12..15) + rope key (m=16) ----
                ssc = psA1.tile([1, c.LOC], F32, name="ss_ps")
                c_ev = []
                for mc in range(c.KVCH):
                    ps = aproj(c.QRCH + mc, get_wa(mc), 2)
                    ev = pAk.tile([128, c.LOC], F32, name=f"c_ev{mc}")
                    nc.scalar.activation(ev[:], ps[:], AF.Copy)
                    c_ev.append(ev)
                    x2 = pAe.tile([128, c.LOC], F32R, name="x2")
                    nc.vector.tensor_mul(x2[:], ev[:], ev[:])
                    nc.tensor.matmul(ssc[:], ones_col_f, x2[:],
                                     start=(mc == 0), stop=(mc == c.KVCH - 1))
                ps = aproj(c.AM - 1, get_wa(c.KVCH), 2)
                kr = pAe.tile([64, c.LOC], F32R, name="kr")
                nc.scalar.activation(kr[:], ps[0:64, :], AF.Copy)
                rps = psX.tile([128, c.LOC], F32, name="aux_ps")
                nc.tensor.matmul(rps[0:64, :], rot_sb[0:64, 0:64], kr[:])
                rk = pAe.tile([64, c.LOC], F32, name="rk")
                nc.vector.tensor_copy(rk[:], rps[0:64, :])
                ra = pAe.tile([64, c.LOC], F32, name="ra")
                rb = pAe.tile([64, c.LOC], F32, name="rb")
                nc.vector.tensor_mul(ra[:], kr[:], cosk[:])
                nc.vector.tensor_mul(rb[:], rk[:], sink[:])
                kro = pAe.tile([64, c.LOC], BF16, name="kro")
                nc.vector.tensor_add(kro[:], ra[:], rb[:])
                nc.gpsimd.dma_start(gin.ap()[c.KVCH * 128:c.GR, :], kro[:])

                t1 = pAe.tile([1, c.LOC], F32, name="rms_t")
                nc.vector.tensor_scalar(
                    t1[:], ssc[:], 1.0 / c.KVR, SA * SA * EPS,
                    mybir.AluOpType.mult, mybir.AluOpType.add)
                st = pAe.tile([1, c.LOC], F32, name="rms_st")
                nc.scalar.activation(st[:], t1[:], AF.Sqrt)
                rsc = pAe.tile([1, c.LOC], F32R, name="rsc")
                with nc.allow_low_precision(reason="f32r for PE broadcast"):
                    nc.vector.reciprocal(rsc[:], st[:])
                bc_ps = psX.tile([128, c.LOC], F32, name="aux_ps")
                nc.tensor.matmul(bc_ps[:], ones_row_f, rsc[:])
                bcc = pAe.tile([128, c.LOC], F32, name="bcc")
                nc.vector.tensor_copy(bcc[:], bc_ps[:])
                cn4 = pAe.tile([128, c.KVCH, c.LOC], BF16, name="cn4")
                for mc in range(c.KVCH):
                    nc.vector.tensor_mul(cn4[:, mc, :], c_ev[mc][:], bcc[:])
                nc.gpsimd.dma_start(
                    gin.ap()[0:c.KVCH * 128, :]
                    .rearrange("(k p) x -> p k x", k=c.KVCH), cn4[:])

                # ---- collective 1: AllGather normalized ckv + rope key ----
                nc.gpsimd.collective_compute(
                    "AllGather", mybir.AluOpType.bypass, replica_groups=GRP,
                    ins=[gin.ap()], outs=[gout.ap()])

                # ---- qa chunks (m=0..11): raw hi/lo, rms applied later --
                ssq = psA1.tile([1, c.LOC], F32, name="ss_ps")
                qah = pp.tile([128, c.QP, 2, c.LOC], FP8, name="qah")
                qal = pp.tile([128, c.QP, 2, c.LOC], FP8, name="qal")
                for m in range(c.QRCH):
                    ps = aproj(m, get_wa(c.KVCH + 1 + m), 2)
                    ev = pAe.tile([128, c.LOC], F32, name="qa_ev")
                    nc.scalar.activation(ev[:], ps[:], AF.Copy, scale=1.0 / SA)
                    x2 = pAe.tile([128, c.LOC], F32R, name="x2")
                    nc.vector.tensor_mul(x2[:], ev[:], ev[:])
                    nc.tensor.matmul(ssq[:], ones_col_f, x2[:],
                                     start=(m == 0), stop=(m == c.QRCH - 1))
                    dst_h = qah[:, m // 2, m % 2, :]
                    nc.vector.tensor_copy(dst_h, ev[:])
                    df = pAe.tile([128, c.LOC], F32, name="df")
                    nc.vector.tensor_sub(df[:], ev[:], dst_h)
                    nc.vector.tensor_copy(qal[:, m // 2, m % 2, :], df[:])
                t2 = pAe.tile([1, c.LOC], F32, name="rms_t2")
                nc.vector.tensor_scalar(
                    t2[:], ssq[:], 1.0 / c.QR, EPS,
                    mybir.AluOpType.mult, mybir.AluOpType.add)
                st2 = pAe.tile([1, c.LOC], F32, name="rms_st2")
                nc.scalar.activation(st2[:], t2[:], AF.Sqrt)
                rsq = pAe.tile([1, c.LOC], F32R, name="rsq")
                with nc.allow_low_precision(reason="f32r for PE broadcast"):
                    nc.vector.reciprocal(rsq[:], st2[:])
                bq_ps = psX.tile([128, c.LOC], F32, name="aux_ps")
                nc.tensor.matmul(bq_ps[:], ones_row_f, rsq[:])
                bcq = pp.tile([128, c.LOC], F32, name="bcq")
                nc.vector.tensor_copy(bcq[:], bq_ps[:])
                cosqn = pp.tile([128, c.LOC], F32, name="cosqn")
                sinqn = pp.tile([128, c.LOC], F32, name="sinqn")
                nc.vector.tensor_mul(cosqn[:], cosq[:], bcq[:])
                nc.vector.tensor_mul(sinqn[:], sinq[:], bcq[:])

            # ================= phase Q: local q b-proj (all heads) ========
            with tc.tile_pool(name="pQ_w", bufs=5) as pQw, \
                 tc.tile_pool(name="pQ_ev", bufs=3) as pQe, \
                 tc.tile_pool(name="pQ_ps", bufs=3, space="PSUM") as psQ, \
                 tc.tile_pool(name="pQ_rot", bufs=1, space="PSUM") as psQr:

                def load_wqb(g, eng=None):
                    w = pQw.tile([128, 4, 2, c.QP, 2, 128], FP8, name="wqb_sb")
                    (eng or nc.sync).dma_start(
                        w[:].rearrange("p o a k i x -> p o (a k i x)"),
                        wqb_d.ap()[g]
                        .rearrange("p o a k i x -> p o (a k i x)"))
                    return w

                wq_tiles = {}
                for gg in range(4):
                    wq_tiles[gg] = load_wqb(gg)

                for g in range(12):
                    if g + 4 < 12:
                        wq_tiles[g + 4] = load_wqb(g + 4)
                    wq = wq_tiles.pop(g)
                    qn4 = None
                    qr2 = None
                    if g < 8:
                        qn4 = pQe.tile([128, 4, 4, 64], BF16, name="qn4")
                    else:
                        qr2 = [pQe.tile([128, 4, 2, 64], BF16,
                                        name=f"qr2_{x}") for x in range(2)]
                    for oo in range(4):
                        o = 4 * g + oo
                        ps = psQ.tile([128, c.LOC], F32, name="psQ")
                        i = 0
                        for (pl, q) in ((0, qah), (0, qal), (1, qah)):
                            for kp in range(c.QP):
                                nc.tensor.matmul(
                                    ps[:], wq[:, oo, pl, kp, :, :],
                                    q[:, kp, :, :],
                                    start=(i == 0), stop=(i == 3 * c.QP - 1),
                                    perf_mode=DR)
                                i += 1
                        if o < c.H:
                            qt = pQe.tile([128, c.LOC], F32, name="qt")
                            nc.scalar.activation(
                                qt[:], ps[:], AF.Copy, scale=1.0 / SQ)
                            nc.vector.tensor_mul(
                                qn4[:, :, oo, :],
                                qt[:].rearrange("p (r x) -> p r x", r=4),
                                bcq[:].rearrange("p (r x) -> p r x", r=4))
                        else:
                            j = o - c.H
                            ro = pQe.tile([128, c.LOC], F32R, name="ro")
                            nc.scalar.activation(
                                ro[:], ps[:], AF.Copy, scale=1.0 / SQ)
                            rps = psQr.tile([128, c.LOC], F32, name="rpsQ")
                            nc.tensor.matmul(rps[:], rot_sb[:], ro[:])
                            rk = pQe.tile([128, c.LOC], F32, name="qrk")
                            nc.vector.tensor_copy(rk[:], rps[:])
                            a = pQe.tile([128, c.LOC], F32, name="qra")
                            b = pQe.tile([128, c.LOC], F32, name="qrb")
                            nc.vector.tensor_mul(a[:], ro[:], cosqn[:])
                            nc.vector.tensor_mul(b[:], rk[:], sinqn[:])
                            nc.vector.tensor_add(
                                qr2[oo // 2][:, :, oo % 2, :],
                                a[:].rearrange("p (r x) -> p r x", r=4),
                                b[:].rearrange("p (r x) -> p r x", r=4))
                    if g < 8:
                        nc.scalar.dma_start(
                            a2a_in.ap()[:, g, :, 0:4, :]
                            .rearrange("r p o x -> p r o x"),
                            qn4[:])
                    else:
                        for x in range(2):
                            d = (g - 8) * 2 + x
                            nc.scalar.dma_start(
                                a2a_in.ap()[:, d, :, 4:6, :]
                                .rearrange("r p s x -> p r s x"),
                                qr2[x][:])

            # ================= shared B/C residents =======================
            # gout reads issue BEFORE the A2A issues: they only wait on the
            # AllGather, so phase B's inputs land while phase Q streams.
            pBC_cm = tc.tile_pool(name="pBC", bufs=1)
            pBC = pBC_cm.__enter__()
            pB_cm = tc.tile_pool(name="pB", bufs=1)
            pB = pB_cm.__enter__()
            knopeT = [pBC.tile([128, c.S], BF16, name=f"knopeT_{m}")
                      for m in range(c.HPC)]
            v_sb = [pBC.tile([128, c.HPC * c.VD], BF16, name=f"v_sb_{ki}")
                    for ki in range(c.NKI)]
            wkbv_sb = pB.tile([128, c.KVCH, 2, 512], BF16, name="wkbv_sb")
            nc.gpsimd.dma_start(
                wkbv_sb[:].rearrange("p k a x -> p (k a x)"),
                wkbv_d.ap().rearrange("p k a x -> p (k a x)"))
            c_T = []
            for kc in range(c.KVCH):
                t = pB.tile([128, c.S], BF16, name=f"c_T_{kc}")
                nc.gpsimd.dma_start(
                    t[:].rearrange("p (g s x) -> p g s x",
                                   g=4, s=N_CORES),
                    gout.ap()[:, kc * 128:(kc + 1) * 128, :]
                    .rearrange("s p (g x) -> p g s x", g=4))
                c_T.append(t)
            for par in range(2):
                nc.gpsimd.dma_start(
                    krope2[par][64 * par:64 * par + 64, :]
                    .rearrange("p (g s x) -> p g s x", g=4, s=N_CORES),
                    gout.ap()[:, c.KVCH * 128:c.GR, :]
                    .rearrange("s p (g x) -> p g s x", g=4))

            # ---- collectives 2..5: AllToAll q^T per 512-col round ----
            for r in range(4):
                nc.gpsimd.collective_compute(
                    "AllToAll", mybir.AluOpType.bypass, replica_groups=GRP,
                    ins=[a2a_in.ap()[r]], outs=[a2a_out.ap()[r]])

            # ================= phase B: kv b-projection ===================
            with tc.tile_pool(name="pB_ps", bufs=3, space="PSUM") as psB:
                for m in range(c.HPC):
                    for n in range(c.S // 512):
                        ps = psB.tile([128, 512], F32, name="psB")
                        for kc in range(c.KVCH):
                            nc.tensor.matmul(
                                ps[:],
                                wkbv_sb[:, kc, 0, m * 128:(m + 1) * 128],
                                c_T[kc][:, n * 512:(n + 1) * 512],
                                start=(kc == 0), stop=(kc == c.KVCH - 1))
                        nc.scalar.activation(
                            knopeT[m][:, n * 512:(n + 1) * 512], ps[:],
                            AF.Copy)
                for ki in range(c.NKI):
                    ps = psB.tile([128, c.HPC * c.VD], F32, name="psB")
                    for kc in range(c.KVCH):
                        nc.tensor.matmul(
                            ps[:], c_T[kc][:, ki * 128:(ki + 1) * 128],
                            wkbv_sb[:, kc, 1, :], start=(kc == 0),
                            stop=(kc == c.KVCH - 1))
                    nc.scalar.activation(v_sb[ki][:], ps[:], AF.Copy)
            pB_cm.__exit__(None, None, None)

            # ================= phase C: attention + out-proj ==============
            with tc.tile_pool(name="pC2", bufs=2) as pC2, \
                 tc.tile_pool(name="pCe", bufs=3) as pCe, \
                 tc.tile_pool(name="pCx", bufs=6) as pCx, \
                 tc.tile_pool(name="pCacc", bufs=2) as pCa, \
                 tc.tile_pool(name="pC_mm", bufs=3, space="PSUM") as psM, \
                 tc.tile_pool(name="pC_sT", bufs=2, space="PSUM") as psT, \
                 tc.tile_pool(name="pC_oT", bufs=2, space="PSUM") as psO, \
                 tc.tile_pool(name="pC_den", bufs=1, space="PSUM") as psD:
                # out-proj weights + causal mask needed only here; SP reaches
                # this point right after the Q weight stream, so these land in
                # the idle DMA window before the first a2a_out arrives.
                nc.sync.dma_start(
                    mask_sb[:].rearrange("p j x -> p (j x)"),
                    mask_d.ap().rearrange("p j x -> p (j x)"))
                nc.sync.dma_start(
                    woh_sb[:].rearrange("p m a b x -> p (m a b x)"),
                    woh_d.ap().rearrange("p m a b x -> p (m a b x)"))
                nc.sync.dma_start(
                    wol_sb[:].rearrange("p m a b x -> p (m a b x)"),
                    wol_d.ap().rearrange("p m a b x -> p (m a b x)"))
                for qi in range(c.NQT):
                    q0 = qi * 512
                    q_all = pC2.tile([128, N_CORES, 6, 64], BF16,
                                     name="q_all")
                    nc.sync.dma_start(
                        q_all[:],
                        a2a_out.ap()[qi].rearrange("s p o x -> p s o x"))
                    qnopeT = [q_all[:, :, h, :] for h in range(c.HPC)]
                    qropeT = [q_all[:, :, 4 + j, :] for j in range(2)]

                    oT8 = [[pC2.tile([128, 2, 512], FP8, name=f"o{x}_{pr}")
                            for pr in range(2)] for x in range(2)]
                    nki = 4 * (qi + 1)

                    def attn_head(h):
                        """ki loop for head h: softmax numerator into PSUM,
                        denominator accumulated on DVE in bf16."""
                        oT_ps = psO.tile([128, 512], F32, name="psO")
                        acc = pCa.tile([128, 512], BF16, name="den_acc")
                        for ki in range(nki):
                            sT_ps = psT.tile([128, 512], F32, name="psT")
                            nc.tensor.matmul(
                                sT_ps[:],
                                knopeT[h][:, ki * 128:(ki + 1) * 128],
                                qnopeT[h], start=True, stop=False)
                            nc.tensor.matmul(
                                sT_ps[:],
                                krope2[h % 2][:, ki * 128:(ki + 1) * 128],
                                qropeT[h // 2], start=False, stop=True)
                            ex = pCx.tile([128, 512], BF16, name="expT")
                            nc.scalar.activation(ex[:], sT_ps[:], AF.Exp)
                            jj = ki - (nki - 4)
                            if jj >= 0:
                                nc.vector.tensor_mul(ex[:], ex[:],
                                                     mask_sb[:, jj, :])
                            if ki == 0:
                                nc.vector.tensor_copy(acc[:], ex[:])
                            else:
                                nc.vector.tensor_add(acc[:], acc[:], ex[:])
                            nc.tensor.matmul(
                                oT_ps[:],
                                v_sb[ki][:, h * c.VD:(h + 1) * c.VD],
                                ex[:], start=(ki == 0), stop=(ki == nki - 1))
                        den_ps = psD.tile([1, 512], F32, name="psD")
                        nc.tensor.matmul(den_ps[:], ones_col_b[:], acc[:],
                                         start=True, stop=True)
                        rec = pCe.tile([1, 512], F32R, name="rec")
                        with nc.allow_low_precision(reason="f32r broadcast"):
                            nc.vector.reciprocal(rec[:], den_ps[:])
                        return oT_ps, rec

                    def norm_head(h, oT_ps, rec):
                        """normalize + fp8 hi/lo split of head h's output.
                        The two fp8 casts run on gpsimd (otherwise idle) to
                        keep DVE off the attention critical path."""
                        bc_ps = psM.tile([128, 512], F32, name="psm")
                        nc.tensor.matmul(bc_ps[:], ones_row_f, rec[:])
                        ov = pCe.tile([128, 512], F32, name="ov")
                        nc.vector.tensor_mul(ov[:], oT_ps[:], bc_ps[:])
                        dst_h = oT8[0][h // 2][:, h % 2, :]
                        nc.gpsimd.tensor_copy(dst_h, ov[:])
                        df = pCe.tile([128, 512], F32, name="odf")
                        nc.vector.tensor_sub(df[:], ov[:], dst_h)
                        nc.gpsimd.tensor_copy(oT8[1][h // 2][:, h % 2, :],
                                              df[:])

                    # software-pipeline: head h's attention runs while head
                    # h-1's reciprocal lands, so the broadcast matmul never
                    # stalls PE.
                    prev = None
                    for h in range(c.HPC):
                        cur = attn_head(h)
                        if prev is not None:
                            norm_head(h - 1, *prev)
                        prev = cur
                    norm_head(c.HPC - 1, *prev)

                    for m4 in range(8):
                        ob4 = pCe.tile([128, 4, 512], BF16, name="ob4")
                        for mm in range(4):
                            m = 4 * m4 + mm
                            ps = psM.tile([128, 512], F32, name="psm")
                            i = 0
                            for pr in range(2):
                                for (w, o8) in ((woh_sb, oT8[0][pr]),
                                                (woh_sb, oT8[1][pr]),
                                                (wol_sb, oT8[0][pr])):
                                    nc.tensor.matmul(
                                        ps[:], w[:, m, pr, :, :], o8[:],
                                        start=(i == 0), stop=(i == 5),
                                        perf_mode=DR)
                                    i += 1
                            nc.scalar.activation(
                                ob4[:, mm, :], ps[:], AF.Copy,
                                scale=1.0 / SWO)
                        nc.scalar.dma_start(
                            out_d.ap()[4 * m4:4 * m4 + 4, :, q0:q0 + 512]
                            .rearrange("m p x -> p m x"), ob4[:])
            pBC_cm.__exit__(None, None, None)
    nc.compile()
    return nc


# --------------------------------------------------------------------------
# public entry point
# --------------------------------------------------------------------------

_CACHED = {}


def _get_nc(cfg):
    if cfg not in _CACHED:
        _CACHED[cfg] = build(cfg)
    return _CACHED[cfg]


def kernel(hidden_states, Wq_a, q_a_ln_w, Wq_b, Wkv_a, kv_a_ln_w, Wkv_b, Wo):
    cfg = FULL
    in_maps = prep_inputs(cfg, hidden_states, Wq_a, q_a_ln_w, Wq_b, Wkv_a,
                          kv_a_ln_w, Wkv_b, Wo)
    nc = _get_nc(cfg)
    res = run_bass_kernel_spmd(nc, in_maps, core_ids=list(range(N_CORES)))
    acc = np.zeros((32, 128, cfg.S), np.float32)
    for r in res.results:
        acc += np.asarray(r["outT"], np.float32)
    out = acc.reshape(cfg.D, cfg.S).T
    return np.ascontiguousarray(out).reshape(1, cfg.S, cfg.D)



# revision 25
# speedup vs baseline: 1.1199x; 1.0159x over previous
"""DeepSeek-V3 MLA forward (B=1, S=2048, D=4096, H=32) on 8 TRN2 NeuronCores.

v2: sequence-sharded low-rank a-projections + in-kernel collectives.

Structure (per core c of 8):
  * Core c owns 256 seq columns: 64-col blocks {512r + 64c : r in 0..3}.
  * Phase A (local): a-projections computed only for the owned columns in
    fp8e4 DoubleRow 3-term hi/lo (error ~1e-3, 0.5 cyc/row). RMS scales
    applied locally; weights pre-scaled by 64/512 into fp8 range with the
    inverse folded into rms (scale-invariant), rope tables, or evacuation
    scales.
  * Normalized ckv + rope key AllGather'ed (2.4MB, ~75us, overlapped).
  * Phase Q (local): q b-projection for ALL 32 heads on owned columns,
    fp8 DoubleRow 3-term, weights streamed from DRAM; rope applied locally.
  * 4 chunked AllToAlls redistribute q^T feature-major to head-owners
    (core d owns heads 4d..4d+3), one per 512-col query tile, pipelined
    against attention.
  * Phase B: kv b-projection from gathered ckv (bf16).
  * Phase C: causal attention with transposed scores + fp8 DoubleRow
    3-term out-projection; host sums the 8 partial out-projections.

Engine budget: PE ~360us busy; evacuations spread DVE/ACT; DMAs batched and
issued from producer engines to avoid SP sequencer head-of-line blocking.
"""

from dataclasses import dataclass

import ml_dtypes
import numpy as np

import concourse.bass as bass
import concourse.mybir as mybir
import concourse.tile as tile
from concourse import bacc
from concourse.bass_utils import run_bass_kernel_spmd

F32 = mybir.dt.float32
F32R = mybir.dt.float32r
BF16 = mybir.dt.bfloat16
FP8 = mybir.dt.float8e4
AF = mybir.ActivationFunctionType
DR = mybir.MatmulPerfMode.DoubleRow
BF16NP = ml_dtypes.bfloat16
E4M3 = ml_dtypes.float8_e4m3

N_CORES = 8
GRP = [[0, 1, 2, 3, 4, 5, 6, 7]]
EPS = 1e-6
THETA = 10000.0

SA = 64.0     # a-proj weight prescale (folded out via rms / rope tables)
SQ = 512.0    # q b-proj weight prescale (folded out at evacuation)
SWO = 64.0    # out-proj weight prescale (folded out at evacuation)


@dataclass(frozen=True)
class Cfg:
    S: int = 2048
    D: int = 4096
    QR: int = 1536
    KVR: int = 512
    H: int = 32
    HPC: int = 4
    NOPE: int = 128
    ROPE: int = 64
    VD: int = 128

    @property
    def DP(self):          # 128x2 contraction pairs in D
        return self.D // 256

    @property
    def QP(self):          # pairs in q lora rank
        return self.QR // 256

    @property
    def QRCH(self):
        return self.QR // 128

    @property
    def KVCH(self):
        return self.KVR // 128

    @property
    def AM(self):          # a-proj out chunks: 12 qa + 4 ckv + 1 rope(pad)
        return self.QRCH + self.KVCH + 1

    @property
    def LOC(self):         # owned columns per core
        return self.S // N_CORES

    @property
    def NQT(self):
        return self.S // 512

    @property
    def NKI(self):
        return self.S // 128

    @property
    def QCH(self):         # q b-proj out chunks: 32 nope + 16 rope
        return self.H + self.H * self.ROPE // 128

    @property
    def GR(self):          # gathered rows: 4*128 ckv + 64 rope key
        return self.KVCH * 128 + self.ROPE


FULL = Cfg()


# --------------------------------------------------------------------------
# host-side input preparation
# --------------------------------------------------------------------------

def _rope_perm(rope):
    return np.concatenate([np.arange(0, rope, 2), np.arange(1, rope, 2)])


def _split8(x):
    hi = x.astype(E4M3)
    lo = (x - hi.astype(np.float32)).astype(E4M3)
    return hi, lo


def prep_inputs(cfg, hidden_states, Wq_a, q_a_ln_w, Wq_b, Wkv_a, kv_a_ln_w,
                Wkv_b, Wo):
    c = cfg
    hs = np.asarray(hidden_states, np.float32).reshape(c.S, c.D)
    Wq_a = np.asarray(Wq_a, np.float32)
    Wq_b = np.asarray(Wq_b, np.float32)
    Wkv_a = np.asarray(Wkv_a, np.float32)
    Wkv_b = np.asarray(Wkv_b, np.float32)
    Wo = np.asarray(Wo, np.float32)
    q_a_ln_w = np.asarray(q_a_ln_w, np.float32)
    kv_a_ln_w = np.asarray(kv_a_ln_w, np.float32)

    nope, rope, vd = c.NOPE, c.ROPE, c.VD
    qd = nope + rope

    # combined a-proj weight (x SA), rope cols permuted, padded to 17*128;
    # hi/lo planes packed per m-chunk: [AM, 128, 2, DP, 2, 128]
    perm_a = _rope_perm(rope)
    Wkv_a_p = np.concatenate(
        [Wkv_a[:, :c.KVR], Wkv_a[:, c.KVR:][:, perm_a]], axis=1)
    wa = np.concatenate([Wq_a, Wkv_a_p], axis=1) * SA      # [D, 2112]
    wa = np.pad(wa, ((0, 0), (0, c.AM * 128 - wa.shape[1])))
    wah_np, wal_np = _split8(wa)

    def _wa_prep(w8):                                      # [AM,128,DP,2,128]
        t = w8.reshape(c.DP, 2, 128, c.AM, 128).transpose(3, 2, 0, 1, 4)
        return np.ascontiguousarray(t)
    wa_hl = np.ascontiguousarray(np.stack(
        [_wa_prep(wah_np), _wa_prep(wal_np)], axis=2))
    # -> [AM, 128, 2, DP, 2, 128]

    # q b-proj weights (x SQ), ln + 1/sqrt(qd) + rope perm folded;
    # out-chunk order: 0..31 = nope of head o; 32+j = rope of heads 2j,2j+1;
    # grouped 4 chunks per DMA: [12, 128, 4, 2, QP, 2, 128]
    scale = qd ** (-0.5)
    wqb_all = (Wq_b * q_a_ln_w[:, None]).reshape(c.QR, c.H, qd) * scale * SQ
    perm = _rope_perm(rope)
    wqb_nope = wqb_all[:, :, :nope]
    wqb_rope = wqb_all[:, :, nope:][:, :, perm]
    cols = [wqb_nope[:, h, :] for h in range(c.H)]
    for j in range(c.H // 2):
        cols.append(np.concatenate(
            [wqb_rope[:, 2 * j, :], wqb_rope[:, 2 * j + 1, :]], axis=1))
    wqb = np.stack(cols, axis=0)                            # [48, QR, 128]
    wqbh_np, wqbl_np = _split8(wqb)

    def _wqb_prep(w8):                                      # [48,128,QP,2,128]
        t = w8.reshape(c.QCH, c.QP, 2, 128, 128).transpose(0, 3, 1, 2, 4)
        return np.ascontiguousarray(t)
    wqb_hl = np.stack([_wqb_prep(wqbh_np), _wqb_prep(wqbl_np)], axis=2)
    # [48, 128, 2, QP, 2, 128] -> [12, 128, 4, 2, QP, 2, 128]
    wqb_hl = np.ascontiguousarray(
        wqb_hl.reshape(12, 4, 128, 2, c.QP, 2, 128).transpose(
            0, 2, 1, 3, 4, 5, 6))

    # kv b-proj weights (bf16), ln folded; packed [128, KVCH, 2, 512]
    wkv_all = (Wkv_b * kv_a_ln_w[:, None]).reshape(c.KVR, c.H, nope + vd)

    # rope tables, feature-major [128, S]
    inv_freq = 1.0 / (THETA ** (np.arange(0, rope, 2, np.float32) / rope))
    freqs = np.outer(np.arange(c.S, dtype=np.float32), inv_freq)
    cosT = np.tile(np.cos(freqs).T, (4, 1)).astype(np.float32)
    sinT = np.tile(np.sin(freqs).T, (4, 1)).astype(np.float32)

    R = np.zeros((128, 128), np.float32)
    for blk in (0, 64):
        for i in range(32):
            R[blk + i, blk + i + 32] = -1.0
            R[blk + i + 32, blk + i] = 1.0
    rotT = np.ascontiguousarray(R.T)

    j = np.arange(4)[:, None, None]
    r = np.arange(128)[None, :, None]
    q = np.arange(512)[None, None, :]
    mask01 = np.ascontiguousarray(
        ((128 * j + r) <= q).astype(BF16NP).transpose(1, 0, 2))  # [128,4,512]

    hsT = hs.T

    in_maps = []
    for core in range(N_CORES):
        own = np.concatenate(
            [np.arange(512 * r + 64 * core, 512 * r + 64 * core + 64)
             for r in range(4)])
        hT_own = hsT[:, own]
        hTh_np, hTl_np = _split8(hT_own)

        def _h_prep(h8):                                    # [128, DP, 2, 256]
            t = h8.reshape(c.DP, 2, 128, c.LOC).transpose(2, 0, 1, 3)
            return np.ascontiguousarray(t)

        hsel = np.arange(core * c.HPC, (core + 1) * c.HPC)
        wkbv = np.empty((c.KVCH, 128, 2, 512), np.float32)
        wkbv[:, :, 0, :] = wkv_all[:, hsel, :nope].reshape(c.KVCH, 128, 512)
        wkbv[:, :, 1, :] = wkv_all[:, hsel, nope:].reshape(c.KVCH, 128, 512)
        wkbv = np.ascontiguousarray(
            wkbv.transpose(1, 0, 2, 3)).astype(BF16NP)      # [128,KVCH,2,512]

        wo_r = Wo.reshape(c.H, vd, c.D)[hsel] * SWO
        wo8 = wo_r.reshape(2, 2, 128, 32, 128).transpose(2, 3, 0, 1, 4)
        woh_np, wol_np = _split8(np.ascontiguousarray(wo8))  # [128,32,2,2,128]

        in_maps.append({
            "hTh": _h_prep(hTh_np), "hTl": _h_prep(hTl_np),
            "wa_hl": wa_hl, "wqb_hl": wqb_hl,
            "wkbv": wkbv, "woh": woh_np, "wol": wol_np,
            "cosq": cosT[:, own].astype(BF16NP),
            "sinq": sinT[:, own].astype(BF16NP),
            "cosk": (cosT[:64, own] / SA).astype(BF16NP),
            "sink": (sinT[:64, own] / SA).astype(BF16NP),
            "rotT": rotT,
            "ones_f": np.ones((128, 128), np.float32),
            "mask01": mask01,
        })
    return in_maps


# --------------------------------------------------------------------------
# kernel builder
# --------------------------------------------------------------------------

def build(cfg):
    c = cfg
    nc = bacc.Bacc("TRN2", target_bir_lowering=False, debug=False,
                   num_devices=N_CORES)

    hTh_d = nc.declare_dram_parameter("hTh", [128, c.DP, 2, c.LOC], FP8, isOutput=False)
    hTl_d = nc.declare_dram_parameter("hTl", [128, c.DP, 2, c.LOC], FP8, isOutput=False)
    wa_d = nc.declare_dram_parameter("wa_hl", [c.AM, 128, 2, c.DP, 2, 128], FP8, isOutput=False)
    wqb_d = nc.declare_dram_parameter("wqb_hl", [12, 128, 4, 2, c.QP, 2, 128], FP8, isOutput=False)
    wkbv_d = nc.declare_dram_parameter("wkbv", [128, c.KVCH, 2, 512], BF16, isOutput=False)
    woh_d = nc.declare_dram_parameter("woh", [128, 32, 2, 2, 128], FP8, isOutput=False)
    wol_d = nc.declare_dram_parameter("wol", [128, 32, 2, 2, 128], FP8, isOutput=False)
    cosq_d = nc.declare_dram_parameter("cosq", [128, c.LOC], BF16, isOutput=False)
    sinq_d = nc.declare_dram_parameter("sinq", [128, c.LOC], BF16, isOutput=False)
    cosk_d = nc.declare_dram_parameter("cosk", [64, c.LOC], BF16, isOutput=False)
    sink_d = nc.declare_dram_parameter("sink", [64, c.LOC], BF16, isOutput=False)
    rot_d = nc.declare_dram_parameter("rotT", [128, 128], F32R, isOutput=False)
    ones_d = nc.declare_dram_parameter("ones_f", [128, 128], F32R, isOutput=False)
    mask_d = nc.declare_dram_parameter("mask01", [128, 4, 512], BF16, isOutput=False)
    out_d = nc.declare_dram_parameter("outT", [32, 128, c.S], BF16, isOutput=True)

    gin = nc.dram_tensor("ckv_gin", [c.GR, c.LOC], BF16)
    gout = nc.dram_tensor("ckv_gout", [N_CORES, c.GR, c.LOC], BF16,
                          addr_space="Shared")
    # [round, dest, partition, chunk, col]: per-(r,p) runs of 4 nope chunks
    # are 512B contiguous, and the phase-C read per qi is one 768B-elem DMA.
    a2a_in = nc.dram_tensor("a2a_in", [4, N_CORES, 128, 6, 64], BF16)
    a2a_out = nc.dram_tensor("a2a_out", [4, N_CORES, 128, 6, 64], BF16)

    with tile.TileContext(nc) as tc:
        with tc.tile_pool(name="persist", bufs=1) as pp:
            cosq = pp.tile([128, c.LOC], BF16, name="cosq")
            sinq = pp.tile([128, c.LOC], BF16, name="sinq")
            cosk = pp.tile([64, c.LOC], BF16, name="cosk")
            sink = pp.tile([64, c.LOC], BF16, name="sink")
            rot_sb = pp.tile([128, 128], F32R, name="rot_sb")
            ones_sb = pp.tile([128, 128], F32R, name="ones_sb")
            hTh = pp.tile([128, c.DP, 2, c.LOC], FP8, name="hTh")
            hTl = pp.tile([128, c.DP, 2, c.LOC], FP8, name="hTl")
            nc.sync.dma_start(hTh[:], hTh_d.ap())
            nc.sync.dma_start(hTl[:], hTl_d.ap())
            for t, d in ((ones_sb, ones_d), (rot_sb, rot_d), (cosk, cosk_d),
                         (sink, sink_d), (cosq, cosq_d), (sinq, sinq_d)):
                nc.scalar.dma_start(t[:], d.ap())
            ones_col_f = ones_sb[:, 0:1]
            ones_row_f = ones_sb[0:1, :]
            ones_col_b = pp.tile([128, 1], BF16, name="ones_col_b")
            nc.vector.memset(ones_col_b[:], 1.0)
            woh_sb = pp.tile([128, 32, 2, 2, 128], FP8, name="woh_sb")
            wol_sb = pp.tile([128, 32, 2, 2, 128], FP8, name="wol_sb")
            mask_sb = pp.tile([128, 4, 512], BF16, name="mask_sb")
            # krope2 memset here so the gather-read into it isn't gated on
            # DVE reaching the end of phase Q.
            krope2 = [pp.tile([128, c.S], BF16, name=f"krope2_{par}")
                      for par in range(2)]
            nc.vector.memset(krope2[0][:], 0.0)
            nc.vector.memset(krope2[1][:], 0.0)

            # small side pool spanning phases A+Q so the first two q b-proj
            # weight loads can slot into the tail of the wa DMA stream.
            pQw0_cm = tc.tile_pool(name="pQ_w0", bufs=2)
            pQw0 = pQw0_cm.__enter__()

            def load_wqb(g, pool):
                w = pool.tile([128, 4, 2, c.QP, 2, 128], FP8, name="wqb_sb")
                nc.sync.dma_start(
                    w[:].rearrange("p o a k i x -> p o (a k i x)"),
                    wqb_d.ap()[g]
                    .rearrange("p o a k i x -> p o (a k i x)"))
                return w

            wq_tiles = {}

            # ================= phase A: local a-projections ===============
            with tc.tile_pool(name="pA_w", bufs=4) as pAw, \
                 tc.tile_pool(name="pA_ev", bufs=4) as pAe, \
                 tc.tile_pool(name="pA_keep", bufs=1) as pAk, \
                 tc.tile_pool(name="pA_ps", bufs=2, space="PSUM") as psA, \
                 tc.tile_pool(name="pA_aux", bufs=1, space="PSUM") as psX, \
                 tc.tile_pool(name="pA_ps1", bufs=1, space="PSUM") as psA1:

                def aproj(m, w_sb, planes):
                    """fp8 hi/lo a-proj for chunk m -> psum [128, LOC]."""
                    ps = psA.tile([128, c.LOC], F32, name="psA")
                    terms = [(0, hTh), (0, hTl)] + ([(1, hTh)] if planes == 2
                                                   else [])
                    i = 0
                    n = len(terms) * c.DP
                    for (pl, h) in terms:
                        for kp in range(c.DP):
                            nc.tensor.matmul(
                                ps[:], w_sb[:, pl, kp, :, :], h[:, kp, :, :],
                                start=(i == 0), stop=(i == n - 1),
                                perf_mode=DR)
                            i += 1
                    return ps

                def load_wa(m):
                    planes = 2
                    w = pAw.tile([128, 2, c.DP, 2, 128], FP8, name="wa_sb")
                    nc.sync.dma_start(
                        w[:, 0:planes].rearrange("p a k i x -> p a (k i x)"),
                        wa_d.ap()[m][:, 0:planes]
                        .rearrange("p a k i x -> p a (k i x)"))
                    return w

                morder = list(range(c.QRCH, c.AM)) + list(range(c.QRCH))
                wa_tiles = {}
                for mm in morder[:3]:
                    wa_tiles[mm] = load_wa(mm)

                def get_wa(idx):
                    m = morder[idx]
                    if idx + 3 < len(morder):
                        wa_tiles[morder[idx + 3]] = load_wa(morder[idx + 3])
                    return wa_tiles.pop(m)

                # ---- ckv chunks (m=12..15) + rope key (m=16) ----
                ssc = psA1.tile([1, c.LOC], F32, name="ss_ps")
                c_ev = []
                for mc in range(c.KVCH):
                    ps = aproj(c.QRCH + mc, get_wa(mc), 2)
                    ev = pAk.tile([128, c.LOC], F32, name=f"c_ev{mc}")
                    nc.scalar.activation(ev[:], ps[:], AF.Copy)
                    c_ev.append(ev)
                    x2 = pAe.tile([128, c.LOC], F32R, name="x2")
                    nc.vector.tensor_mul(x2[:], ev[:], ev[:])
                    nc.tensor.matmul(ssc[:], ones_col_f, x2[:],
                                     start=(mc == 0), stop=(mc == c.KVCH - 1))
                ps = aproj(c.AM - 1, get_wa(c.KVCH), 2)
                kr = pAe.tile([64, c.LOC], F32R, name="kr")
                nc.scalar.activation(kr[:], ps[0:64, :], AF.Copy)
                rps = psX.tile([128, c.LOC], F32, name="aux_ps")
                nc.tensor.matmul(rps[0:64, :], rot_sb[0:64, 0:64], kr[:])
                rk = pAe.tile([64, c.LOC], F32, name="rk")
                nc.vector.tensor_copy(rk[:], rps[0:64, :])
                ra = pAe.tile([64, c.LOC], F32, name="ra")
                rb = pAe.tile([64, c.LOC], F32, name="rb")
                nc.vector.tensor_mul(ra[:], kr[:], cosk[:])
                nc.vector.tensor_mul(rb[:], rk[:], sink[:])
                kro = pAe.tile([64, c.LOC], BF16, name="kro")
                nc.vector.tensor_add(kro[:], ra[:], rb[:])
                nc.gpsimd.dma_start(gin.ap()[c.KVCH * 128:c.GR, :], kro[:])

                t1 = pAe.tile([1, c.LOC], F32, name="rms_t")
                nc.vector.tensor_scalar(
                    t1[:], ssc[:], 1.0 / c.KVR, SA * SA * EPS,
                    mybir.AluOpType.mult, mybir.AluOpType.add)
                st = pAe.tile([1, c.LOC], F32, name="rms_st")
                nc.scalar.activation(st[:], t1[:], AF.Sqrt)
                rsc = pAe.tile([1, c.LOC], F32R, name="rsc")
                with nc.allow_low_precision(reason="f32r for PE broadcast"):
                    nc.vector.reciprocal(rsc[:], st[:])
                bc_ps = psX.tile([128, c.LOC], F32, name="aux_ps")
                nc.tensor.matmul(bc_ps[:], ones_row_f, rsc[:])
                bcc = pAe.tile([128, c.LOC], F32, name="bcc")
                nc.vector.tensor_copy(bcc[:], bc_ps[:])
                cn4 = pAe.tile([128, c.KVCH, c.LOC], BF16, name="cn4")
                for mc in range(c.KVCH):
                    nc.vector.tensor_mul(cn4[:, mc, :], c_ev[mc][:], bcc[:])
                nc.gpsimd.dma_start(
                    gin.ap()[0:c.KVCH * 128, :]
                    .rearrange("(k p) x -> p k x", k=c.KVCH), cn4[:])

                # ---- collective 1: AllGather normalized ckv + rope key ----
                nc.gpsimd.collective_compute(
                    "AllGather", mybir.AluOpType.bypass, replica_groups=GRP,
                    ins=[gin.ap()], outs=[gout.ap()])

                # ---- qa chunks (m=0..11): raw hi/lo, rms applied later --
                ssq = psA1.tile([1, c.LOC], F32, name="ss_ps")
                qah = pp.tile([128, c.QP, 2, c.LOC], FP8, name="qah")
                qal = pp.tile([128, c.QP, 2, c.LOC], FP8, name="qal")
                for m in range(c.QRCH):
                    if m in (9, 10):
                        wq_tiles[m - 9] = load_wqb(m - 9, pQw0)
                    ps = aproj(m, get_wa(c.KVCH + 1 + m), 2)
                    ev = pAe.tile([128, c.LOC], F32, name="qa_ev")
                    nc.scalar.activation(ev[:], ps[:], AF.Copy, scale=1.0 / SA)
                    x2 = pAe.tile([128, c.LOC], F32R, name="x2")
                    nc.vector.tensor_mul(x2[:], ev[:], ev[:])
                    nc.tensor.matmul(ssq[:], ones_col_f, x2[:],
                                     start=(m == 0), stop=(m == c.QRCH - 1))
                    dst_h = qah[:, m // 2, m % 2, :]
                    nc.vector.tensor_copy(dst_h, ev[:])
                    df = pAe.tile([128, c.LOC], F32, name="df")
                    nc.vector.tensor_sub(df[:], ev[:], dst_h)
                    nc.vector.tensor_copy(qal[:, m // 2, m % 2, :], df[:])
                t2 = pAe.tile([1, c.LOC], F32, name="rms_t2")
                nc.vector.tensor_scalar(
                    t2[:], ssq[:], 1.0 / c.QR, EPS,
                    mybir.AluOpType.mult, mybir.AluOpType.add)
                st2 = pAe.tile([1, c.LOC], F32, name="rms_st2")
                nc.scalar.activation(st2[:], t2[:], AF.Sqrt)
                rsq = pAe.tile([1, c.LOC], F32R, name="rsq")
                with nc.allow_low_precision(reason="f32r for PE broadcast"):
                    nc.vector.reciprocal(rsq[:], st2[:])
                bq_ps = psX.tile([128, c.LOC], F32, name="aux_ps")
                nc.tensor.matmul(bq_ps[:], ones_row_f, rsq[:])
                bcq = pp.tile([128, c.LOC], F32, name="bcq")
                nc.vector.tensor_copy(bcq[:], bq_ps[:])
                cosqn = pp.tile([128, c.LOC], F32, name="cosqn")
                sinqn = pp.tile([128, c.LOC], F32, name="sinqn")
                nc.vector.tensor_mul(cosqn[:], cosq[:], bcq[:])
                nc.vector.tensor_mul(sinqn[:], sinq[:], bcq[:])

            # ================= phase Q: local q b-proj (all heads) ========
            with tc.tile_pool(name="pQ_w", bufs=4) as pQw, \
                 tc.tile_pool(name="pQ_ev", bufs=3) as pQe, \
                 tc.tile_pool(name="pQ_ps", bufs=3, space="PSUM") as psQ, \
                 tc.tile_pool(name="pQ_rot", bufs=1, space="PSUM") as psQr:

                for gg in range(2, 5):
                    wq_tiles[gg] = load_wqb(gg, pQw)

                for g in range(12):
                    if g + 5 < 12:
                        wq_tiles[g + 5] = load_wqb(g + 5, pQw)
                    wq = wq_tiles.pop(g)
                    qn4 = None
                    qr2 = None
                    if g < 8:
                        qn4 = pQe.tile([128, 4, 4, 64], BF16, name="qn4")
                    else:
                        qr2 = [pQe.tile([128, 4, 2, 64], BF16,
                                        name=f"qr2_{x}") for x in range(2)]
                    for oo in range(4):
                        o = 4 * g + oo
                        ps = psQ.tile([128, c.LOC], F32, name="psQ")
                        i = 0
                        for (pl, q) in ((0, qah), (0, qal), (1, qah)):
                            for kp in range(c.QP):
                                nc.tensor.matmul(
                                    ps[:], wq[:, oo, pl, kp, :, :],
                                    q[:, kp, :, :],
                                    start=(i == 0), stop=(i == 3 * c.QP - 1),
                                    perf_mode=DR)
                                i += 1
                        if o < c.H:
                            qt = pQe.tile([128, c.LOC], F32, name="qt")
                            nc.scalar.activation(
                                qt[:], ps[:], AF.Copy, scale=1.0 / SQ)
                            nc.vector.tensor_mul(
                                qn4[:, :, oo, :],
                                qt[:].rearrange("p (r x) -> p r x", r=4),
                                bcq[:].rearrange("p (r x) -> p r x", r=4))
                        else:
                            j = o - c.H
                            ro = pQe.tile([128, c.LOC], F32R, name="ro")
                            nc.scalar.activation(
                                ro[:], ps[:], AF.Copy, scale=1.0 / SQ)
                            rps = psQr.tile([128, c.LOC], F32, name="rpsQ")
                            nc.tensor.matmul(rps[:], rot_sb[:], ro[:])
                            rk = pQe.tile([128, c.LOC], F32, name="qrk")
                            nc.vector.tensor_copy(rk[:], rps[:])
                            a = pQe.tile([128, c.LOC], F32, name="qra")
                            b = pQe.tile([128, c.LOC], F32, name="qrb")
                            nc.vector.tensor_mul(a[:], ro[:], cosqn[:])
                            nc.vector.tensor_mul(b[:], rk[:], sinqn[:])
                            nc.vector.tensor_add(
                                qr2[oo // 2][:, :, oo % 2, :],
                                a[:].rearrange("p (r x) -> p r x", r=4),
                                b[:].rearrange("p (r x) -> p r x", r=4))
                    if g < 8:
                        nc.sync.dma_start(
                            a2a_in.ap()[:, g, :, 0:4, :]
                            .rearrange("r p o x -> p r o x"),
                            qn4[:])
                    else:
                        for x in range(2):
                            d = (g - 8) * 2 + x
                            nc.sync.dma_start(
                                a2a_in.ap()[:, d, :, 4:6, :]
                                .rearrange("r p s x -> p r s x"),
                                qr2[x][:])

            pQw0_cm.__exit__(None, None, None)

            # ================= shared B/C residents =======================
            # gout reads issue BEFORE the A2A issues: they only wait on the
            # AllGather, so phase B's inputs land while phase Q streams.
            pBC_cm = tc.tile_pool(name="pBC", bufs=1)
            pBC = pBC_cm.__enter__()
            pB_cm = tc.tile_pool(name="pB", bufs=1)
            pB = pB_cm.__enter__()
            knopeT = [pBC.tile([128, c.S], BF16, name=f"knopeT_{m}")
                      for m in range(c.HPC)]
            v_sb = [pBC.tile([128, c.HPC * c.VD], BF16, name=f"v_sb_{ki}")
                    for ki in range(c.NKI)]
            wkbv_sb = pB.tile([128, c.KVCH, 2, 512], BF16, name="wkbv_sb")
            nc.gpsimd.dma_start(
                wkbv_sb[:].rearrange("p k a x -> p (k a x)"),
                wkbv_d.ap().rearrange("p k a x -> p (k a x)"))
            c_T = []
            for kc in range(c.KVCH):
                t = pB.tile([128, c.S], BF16, name=f"c_T_{kc}")
                nc.gpsimd.dma_start(
                    t[:].rearrange("p (g s x) -> p g s x",
                                   g=4, s=N_CORES),
                    gout.ap()[:, kc * 128:(kc + 1) * 128, :]
                    .rearrange("s p (g x) -> p g s x", g=4))
                c_T.append(t)
            for par in range(2):
                nc.gpsimd.dma_start(
                    krope2[par][64 * par:64 * par + 64, :]
                    .rearrange("p (g s x) -> p g s x", g=4, s=N_CORES),
                    gout.ap()[:, c.KVCH * 128:c.GR, :]
                    .rearrange("s p (g x) -> p g s x", g=4))

            # ---- collectives 2..5: AllToAll q^T per 512-col round ----
            for r in range(4):
                nc.gpsimd.collective_compute(
                    "AllToAll", mybir.AluOpType.bypass, replica_groups=GRP,
                    ins=[a2a_in.ap()[r]], outs=[a2a_out.ap()[r]])

            # ================= phase B: kv b-projection ===================
            with tc.tile_pool(name="pB_ps", bufs=3, space="PSUM") as psB:
                for m in range(c.HPC):
                    for n in range(c.S // 512):
                        ps = psB.tile([128, 512], F32, name="psB")
                        for kc in range(c.KVCH):
                            nc.tensor.matmul(
                                ps[:],
                                wkbv_sb[:, kc, 0, m * 128:(m + 1) * 128],
                                c_T[kc][:, n * 512:(n + 1) * 512],
                                start=(kc == 0), stop=(kc == c.KVCH - 1))
                        nc.scalar.activation(
                            knopeT[m][:, n * 512:(n + 1) * 512], ps[:],
                            AF.Copy)
                for ki in range(c.NKI):
                    ps = psB.tile([128, c.HPC * c.VD], F32, name="psB")
                    for kc in range(c.KVCH):
                        nc.tensor.matmul(
                            ps[:], c_T[kc][:, ki * 128:(ki + 1) * 128],
                            wkbv_sb[:, kc, 1, :], start=(kc == 0),
                            stop=(kc == c.KVCH - 1))
                    nc.scalar.activation(v_sb[ki][:], ps[:], AF.Copy)
            pB_cm.__exit__(None, None, None)

            # ================= phase C: attention + out-proj ==============
            with tc.tile_pool(name="pC2", bufs=2) as pC2, \
                 tc.tile_pool(name="pCe", bufs=3) as pCe, \
                 tc.tile_pool(name="pCx", bufs=6) as pCx, \
                 tc.tile_pool(name="pCacc", bufs=2) as pCa, \
                 tc.tile_pool(name="pC_mm", bufs=3, space="PSUM") as psM, \
                 tc.tile_pool(name="pC_sT", bufs=2, space="PSUM") as psT, \
                 tc.tile_pool(name="pC_oT", bufs=2, space="PSUM") as psO, \
                 tc.tile_pool(name="pC_den", bufs=1, space="PSUM") as psD:
                # out-proj weights + causal mask needed only here; SP reaches
                # this point right after the Q weight stream, so these land in
                # the idle DMA window before the first a2a_out arrives.
                nc.sync.dma_start(
                    mask_sb[:].rearrange("p j x -> p (j x)"),
                    mask_d.ap().rearrange("p j x -> p (j x)"))
                nc.sync.dma_start(
                    woh_sb[:].rearrange("p m a b x -> p (m a b x)"),
                    woh_d.ap().rearrange("p m a b x -> p (m a b x)"))
                nc.sync.dma_start(
                    wol_sb[:].rearrange("p m a b x -> p (m a b x)"),
                    wol_d.ap().rearrange("p m a b x -> p (m a b x)"))
                for qi in range(c.NQT):
                    q0 = qi * 512
                    q_all = pC2.tile([128, N_CORES, 6, 64], BF16,
                                     name="q_all")
                    nc.sync.dma_start(
                        q_all[:],
                        a2a_out.ap()[qi].rearrange("s p o x -> p s o x"))
                    qnopeT = [q_all[:, :, h, :] for h in range(c.HPC)]
                    qropeT = [q_all[:, :, 4 + j, :] for j in range(2)]

                    oT8 = [[pC2.tile([128, 2, 512], FP8, name=f"o{x}_{pr}")
                            for pr in range(2)] for x in range(2)]
                    nki = 4 * (qi + 1)

                    def attn_head(h):
                        """ki loop for head h: softmax numerator into PSUM,
                        denominator accumulated on DVE in bf16."""
                        oT_ps = psO.tile([128, 512], F32, name="psO")
                        acc = pCa.tile([128, 512], BF16, name="den_acc")
                        for ki in range(nki):
                            sT_ps = psT.tile([128, 512], F32, name="psT")
                            nc.tensor.matmul(
                                sT_ps[:],
                                knopeT[h][:, ki * 128:(ki + 1) * 128],
                                qnopeT[h], start=True, stop=False)
                            nc.tensor.matmul(
                                sT_ps[:],
                                krope2[h % 2][:, ki * 128:(ki + 1) * 128],
                                qropeT[h // 2], start=False, stop=True)
                            ex = pCx.tile([128, 512], BF16, name="expT")
                            nc.scalar.activation(ex[:], sT_ps[:], AF.Exp)
                            jj = ki - (nki - 4)
                            if jj >= 0:
                                nc.vector.tensor_mul(ex[:], ex[:],
                                                     mask_sb[:, jj, :])
                            if ki == 0:
                                nc.vector.tensor_copy(acc[:], ex[:])
                            else:
                                nc.vector.tensor_add(acc[:], acc[:], ex[:])
                            nc.tensor.matmul(
                                oT_ps[:],
                                v_sb[ki][:, h * c.VD:(h + 1) * c.VD],
                                ex[:], start=(ki == 0), stop=(ki == nki - 1))
                        den_ps = psD.tile([1, 512], F32, name="psD")
                        nc.tensor.matmul(den_ps[:], ones_col_b[:], acc[:],
                                         start=True, stop=True)
                        rec = pCe.tile([1, 512], F32R, name="rec")
                        with nc.allow_low_precision(reason="f32r broadcast"):
                            nc.vector.reciprocal(rec[:], den_ps[:])
                        return oT_ps, rec

                    def norm_head(h, oT_ps, rec):
                        """normalize + fp8 hi/lo split of head h's output.
                        The two fp8 casts run on gpsimd (otherwise idle) to
                        keep DVE off the attention critical path."""
                        bc_ps = psM.tile([128, 512], F32, name="psm")
                        nc.tensor.matmul(bc_ps[:], ones_row_f, rec[:])
                        ov = pCe.tile([128, 512], F32, name="ov")
                        nc.vector.tensor_mul(ov[:], oT_ps[:], bc_ps[:])
                        dst_h = oT8[0][h // 2][:, h % 2, :]
                        nc.gpsimd.tensor_copy(dst_h, ov[:])
                        df = pCe.tile([128, 512], F32, name="odf")
                        nc.vector.tensor_sub(df[:], ov[:], dst_h)
                        nc.gpsimd.tensor_copy(oT8[1][h // 2][:, h % 2, :],
                                              df[:])

                    # software-pipeline: head h's attention runs while head
                    # h-1's reciprocal lands, so the broadcast matmul never
                    # stalls PE.
                    prev = None
                    for h in range(c.HPC):
                        cur = attn_head(h)
                        if prev is not None:
                            norm_head(h - 1, *prev)
                        prev = cur
                    norm_head(c.HPC - 1, *prev)

                    for m4 in range(8):
                        ob4 = pCe.tile([128, 4, 512], BF16, name="ob4")
                        for mm in range(4):
                            m = 4 * m4 + mm
                            ps = psM.tile([128, 512], F32, name="psm")
                            i = 0
                            for pr in range(2):
                                for (w, o8) in ((woh_sb, oT8[0][pr]),
                                                (woh_sb, oT8[1][pr]),
                                                (wol_sb, oT8[0][pr])):
                                    nc.tensor.matmul(
                                        ps[:], w[:, m, pr, :, :], o8[:],
                                        start=(i == 0), stop=(i == 5),
                                        perf_mode=DR)
                                    i += 1
                            nc.scalar.activation(
                                ob4[:, mm, :], ps[:], AF.Copy,
                                scale=1.0 / SWO)
                        nc.scalar.dma_start(
                            out_d.ap()[4 * m4:4 * m4 + 4, :, q0:q0 + 512]
                            .rearrange("m p x -> p m x"), ob4[:])
            pBC_cm.__exit__(None, None, None)
    nc.compile()
    return nc


# --------------------------------------------------------------------------
# public entry point
# --------------------------------------------------------------------------

_CACHED = {}


def _get_nc(cfg):
    if cfg not in _CACHED:
        _CACHED[cfg] = build(cfg)
    return _CACHED[cfg]


def kernel(hidden_states, Wq_a, q_a_ln_w, Wq_b, Wkv_a, kv_a_ln_w, Wkv_b, Wo):
    cfg = FULL
    in_maps = prep_inputs(cfg, hidden_states, Wq_a, q_a_ln_w, Wq_b, Wkv_a,
                          kv_a_ln_w, Wkv_b, Wo)
    nc = _get_nc(cfg)
    res = run_bass_kernel_spmd(nc, in_maps, core_ids=list(range(N_CORES)))
    acc = np.zeros((32, 128, cfg.S), np.float32)
    for r in res.results:
        acc += np.asarray(r["outT"], np.float32)
    out = acc.reshape(cfg.D, cfg.S).T
    return np.ascontiguousarray(out).reshape(1, cfg.S, cfg.D)



# revision 32
# speedup vs baseline: 1.1256x; 1.0052x over previous
"""DeepSeek-V3 MLA forward (B=1, S=2048, D=4096, H=32) on 8 TRN2 NeuronCores.

v2: sequence-sharded low-rank a-projections + in-kernel collectives.

Structure (per core c of 8):
  * Core c owns 256 seq columns: 64-col blocks {512r + 64c : r in 0..3}.
  * Phase A (local): a-projections computed only for the owned columns in
    fp8e4 DoubleRow 3-term hi/lo (error ~1e-3, 0.5 cyc/row). RMS scales
    applied locally; weights pre-scaled by 64/512 into fp8 range with the
    inverse folded into rms (scale-invariant), rope tables, or evacuation
    scales.
  * Normalized ckv + rope key AllGather'ed (2.4MB, ~75us, overlapped).
  * Phase Q (local): q b-projection for ALL 32 heads on owned columns,
    fp8 DoubleRow 3-term, weights streamed from DRAM; rope applied locally.
  * 4 chunked AllToAlls redistribute q^T feature-major to head-owners
    (core d owns heads 4d..4d+3), one per 512-col query tile, pipelined
    against attention.
  * Phase B: kv b-projection from gathered ckv (bf16).
  * Phase C: causal attention with transposed scores + fp8 DoubleRow
    3-term out-projection; host sums the 8 partial out-projections.

Engine budget: PE ~360us busy; evacuations spread DVE/ACT; DMAs batched and
issued from producer engines to avoid SP sequencer head-of-line blocking.
"""

from dataclasses import dataclass

import ml_dtypes
import numpy as np

import concourse.bass as bass
import concourse.mybir as mybir
import concourse.tile as tile
from concourse import bacc
from concourse.bass_utils import run_bass_kernel_spmd

F32 = mybir.dt.float32
F32R = mybir.dt.float32r
BF16 = mybir.dt.bfloat16
FP8 = mybir.dt.float8e4
AF = mybir.ActivationFunctionType
DR = mybir.MatmulPerfMode.DoubleRow
BF16NP = ml_dtypes.bfloat16
E4M3 = ml_dtypes.float8_e4m3

N_CORES = 8
GRP = [[0, 1, 2, 3, 4, 5, 6, 7]]
EPS = 1e-6
THETA = 10000.0

SA = 64.0     # a-proj weight prescale (folded out via rms / rope tables)
SQ = 512.0    # q b-proj weight prescale (folded out at evacuation)
SWO = 64.0    # out-proj weight prescale (folded out at evacuation)
SKV = 64.0    # kv b-proj weight prescale (folded out at evacuation)


@dataclass(frozen=True)
class Cfg:
    S: int = 2048
    D: int = 4096
    QR: int = 1536
    KVR: int = 512
    H: int = 32
    HPC: int = 4
    NOPE: int = 128
    ROPE: int = 64
    VD: int = 128

    @property
    def DP(self):          # 128x2 contraction pairs in D
        return self.D // 256

    @property
    def QP(self):          # pairs in q lora rank
        return self.QR // 256

    @property
    def QRCH(self):
        return self.QR // 128

    @property
    def KVCH(self):
        return self.KVR // 128

    @property
    def AM(self):          # a-proj out chunks: 12 qa + 4 ckv + 1 rope(pad)
        return self.QRCH + self.KVCH + 1

    @property
    def LOC(self):         # owned columns per core
        return self.S // N_CORES

    @property
    def NQT(self):
        return self.S // 512

    @property
    def NKI(self):
        return self.S // 128

    @property
    def QCH(self):         # q b-proj out chunks: 32 nope + 16 rope
        return self.H + self.H * self.ROPE // 128

    @property
    def GR(self):          # gathered rows: 4*128 ckv + 64 rope key
        return self.KVCH * 128 + self.ROPE


FULL = Cfg()


# --------------------------------------------------------------------------
# host-side input preparation
# --------------------------------------------------------------------------

def _rope_perm(rope):
    return np.concatenate([np.arange(0, rope, 2), np.arange(1, rope, 2)])


def _split8(x):
    hi = x.astype(E4M3)
    lo = (x - hi.astype(np.float32)).astype(E4M3)
    return hi, lo


def prep_inputs(cfg, hidden_states, Wq_a, q_a_ln_w, Wq_b, Wkv_a, kv_a_ln_w,
                Wkv_b, Wo):
    c = cfg
    hs = np.asarray(hidden_states, np.float32).reshape(c.S, c.D)
    Wq_a = np.asarray(Wq_a, np.float32)
    Wq_b = np.asarray(Wq_b, np.float32)
    Wkv_a = np.asarray(Wkv_a, np.float32)
    Wkv_b = np.asarray(Wkv_b, np.float32)
    Wo = np.asarray(Wo, np.float32)
    q_a_ln_w = np.asarray(q_a_ln_w, np.float32)
    kv_a_ln_w = np.asarray(kv_a_ln_w, np.float32)

    nope, rope, vd = c.NOPE, c.ROPE, c.VD
    qd = nope + rope

    # combined a-proj weight (x SA), rope cols permuted, padded to 17*128;
    # hi/lo planes packed per m-chunk: [AM, 128, 2, DP, 2, 128]
    perm_a = _rope_perm(rope)
    Wkv_a_p = np.concatenate(
        [Wkv_a[:, :c.KVR], Wkv_a[:, c.KVR:][:, perm_a]], axis=1)
    wa = np.concatenate([Wq_a, Wkv_a_p], axis=1) * SA      # [D, 2112]
    wa = np.pad(wa, ((0, 0), (0, c.AM * 128 - wa.shape[1])))
    wah_np, wal_np = _split8(wa)

    def _wa_prep(w8):                                      # [AM,128,DP,2,128]
        t = w8.reshape(c.DP, 2, 128, c.AM, 128).transpose(3, 2, 0, 1, 4)
        return np.ascontiguousarray(t)
    wa_hl = np.ascontiguousarray(np.stack(
        [_wa_prep(wah_np), _wa_prep(wal_np)], axis=2))
    # -> [AM, 128, 2, DP, 2, 128]

    # q b-proj weights (x SQ), ln + 1/sqrt(qd) + rope perm folded;
    # out-chunk order: 0..31 = nope of head o; 32+j = rope of heads 2j,2j+1;
    # grouped 4 chunks per DMA: [12, 128, 4, 2, QP, 2, 128]
    scale = qd ** (-0.5)
    wqb_all = (Wq_b * q_a_ln_w[:, None]).reshape(c.QR, c.H, qd) * scale * SQ
    perm = _rope_perm(rope)
    wqb_nope = wqb_all[:, :, :nope]
    wqb_rope = wqb_all[:, :, nope:][:, :, perm]
    cols = [wqb_nope[:, h, :] for h in range(c.H)]
    for j in range(c.H // 2):
        cols.append(np.concatenate(
            [wqb_rope[:, 2 * j, :], wqb_rope[:, 2 * j + 1, :]], axis=1))
    wqb = np.stack(cols, axis=0)                            # [48, QR, 128]
    wqbh_np, wqbl_np = _split8(wqb)

    def _wqb_prep(w8):                                      # [48,128,QP,2,128]
        t = w8.reshape(c.QCH, c.QP, 2, 128, 128).transpose(0, 3, 1, 2, 4)
        return np.ascontiguousarray(t)
    wqb_hl = np.stack([_wqb_prep(wqbh_np), _wqb_prep(wqbl_np)], axis=2)
    # [48, 128, 2, QP, 2, 128] -> [12, 128, 4, 2, QP, 2, 128]
    wqb_hl = np.ascontiguousarray(
        wqb_hl.reshape(12, 4, 128, 2, c.QP, 2, 128).transpose(
            0, 2, 1, 3, 4, 5, 6))

    # kv b-proj weights (bf16), ln folded; packed [128, KVCH, 2, 512]
    wkv_all = (Wkv_b * kv_a_ln_w[:, None]).reshape(c.KVR, c.H, nope + vd)

    # rope tables, feature-major [128, S]
    inv_freq = 1.0 / (THETA ** (np.arange(0, rope, 2, np.float32) / rope))
    freqs = np.outer(np.arange(c.S, dtype=np.float32), inv_freq)
    cosT = np.tile(np.cos(freqs).T, (4, 1)).astype(np.float32)
    sinT = np.tile(np.sin(freqs).T, (4, 1)).astype(np.float32)

    R = np.zeros((128, 128), np.float32)
    for blk in (0, 64):
        for i in range(32):
            R[blk + i, blk + i + 32] = -1.0
            R[blk + i + 32, blk + i] = 1.0
    rotT = np.ascontiguousarray(R.T)

    j = np.arange(4)[:, None, None]
    r = np.arange(128)[None, :, None]
    q = np.arange(512)[None, None, :]
    mask01 = np.ascontiguousarray(
        ((128 * j + r) <= q).astype(BF16NP).transpose(1, 0, 2))  # [128,4,512]

    hsT = hs.T

    in_maps = []
    for core in range(N_CORES):
        own = np.concatenate(
            [np.arange(512 * r + 64 * core, 512 * r + 64 * core + 64)
             for r in range(4)])
        hT_own = hsT[:, own]
        hTh_np, hTl_np = _split8(hT_own)

        def _h_prep(h8):                                    # [128, DP, 2, 256]
            t = h8.reshape(c.DP, 2, 128, c.LOC).transpose(2, 0, 1, 3)
            return np.ascontiguousarray(t)

        hsel = np.arange(core * c.HPC, (core + 1) * c.HPC)
        # fp8 hi/lo kv b-proj weights [128, pl, kp, i, sec, 512] with
        # contraction index r = kp*256 + i*128 + p (DoubleRow pairing)
        wn = wkv_all[:, hsel, :nope].reshape(c.KVR, 512) * SKV
        wv = wkv_all[:, hsel, nope:].reshape(c.KVR, 512) * SKV
        wsec = np.stack([wn, wv], axis=1)                   # [KVR, 2, 512]
        wsec = wsec.reshape(2, 2, 128, 2, 512).transpose(2, 0, 1, 3, 4)
        # -> [128(p), kp, i, sec, 512]
        w8h, w8l = _split8(wsec)
        wkbv = np.ascontiguousarray(
            np.stack([w8h, w8l], axis=1))       # [128, pl, kp, i, sec, 512]

        wo_r = Wo.reshape(c.H, vd, c.D)[hsel] * SWO
        wo8 = wo_r.reshape(2, 2, 128, 32, 128).transpose(2, 3, 0, 1, 4)
        woh_np, wol_np = _split8(np.ascontiguousarray(wo8))  # [128,32,2,2,128]

        in_maps.append({
            "hTh": _h_prep(hTh_np), "hTl": _h_prep(hTl_np),
            "wa_hl": wa_hl, "wqb_hl": wqb_hl,
            "wkbv": wkbv, "woh": woh_np, "wol": wol_np,
            "cosq": cosT[:, own].astype(BF16NP),
            "sinq": sinT[:, own].astype(BF16NP),
            "cosk": (cosT[:64, own] / SA).astype(BF16NP),
            "sink": (sinT[:64, own] / SA).astype(BF16NP),
            "rotT": rotT,
            "ones_f": np.ones((128, 128), np.float32),
            "mask01": mask01,
        })
    return in_maps


# --------------------------------------------------------------------------
# kernel builder
# --------------------------------------------------------------------------

def build(cfg):
    c = cfg
    nc = bacc.Bacc("TRN2", target_bir_lowering=False, debug=False,
                   num_devices=N_CORES)

    hTh_d = nc.declare_dram_parameter("hTh", [128, c.DP, 2, c.LOC], FP8, isOutput=False)
    hTl_d = nc.declare_dram_parameter("hTl", [128, c.DP, 2, c.LOC], FP8, isOutput=False)
    wa_d = nc.declare_dram_parameter("wa_hl", [c.AM, 128, 2, c.DP, 2, 128], FP8, isOutput=False)
    wqb_d = nc.declare_dram_parameter("wqb_hl", [12, 128, 4, 2, c.QP, 2, 128], FP8, isOutput=False)
    wkbv_d = nc.declare_dram_parameter("wkbv", [128, 2, 2, 2, 2, 512], FP8, isOutput=False)
    woh_d = nc.declare_dram_parameter("woh", [128, 32, 2, 2, 128], FP8, isOutput=False)
    wol_d = nc.declare_dram_parameter("wol", [128, 32, 2, 2, 128], FP8, isOutput=False)
    cosq_d = nc.declare_dram_parameter("cosq", [128, c.LOC], BF16, isOutput=False)
    sinq_d = nc.declare_dram_parameter("sinq", [128, c.LOC], BF16, isOutput=False)
    cosk_d = nc.declare_dram_parameter("cosk", [64, c.LOC], BF16, isOutput=False)
    sink_d = nc.declare_dram_parameter("sink", [64, c.LOC], BF16, isOutput=False)
    rot_d = nc.declare_dram_parameter("rotT", [128, 128], F32R, isOutput=False)
    ones_d = nc.declare_dram_parameter("ones_f", [128, 128], F32R, isOutput=False)
    mask_d = nc.declare_dram_parameter("mask01", [128, 4, 512], BF16, isOutput=False)
    out_d = nc.declare_dram_parameter("outT", [32, 128, c.S], BF16, isOutput=True)

    # gather split in two: the ckv part (fp8 hi/lo planes) issues ~30us and
    # unblocks phase B's reads early; the small rope-key gather follows.
    gin_c = nc.dram_tensor("ckv_gin", [2, c.KVCH, 128, c.LOC], FP8)
    gout_c = nc.dram_tensor("ckv_gout", [N_CORES, 2, c.KVCH, 128, c.LOC],
                            FP8, addr_space="Shared")
    gin_k = nc.dram_tensor("kr_gin", [64, c.LOC], BF16)
    gout_k = nc.dram_tensor("kr_gout", [N_CORES, 64, c.LOC], BF16,
                            addr_space="Shared")
    # [round, dest, partition, chunk, col]: per-(r,p) runs of 4 nope chunks
    # are 512B contiguous, and the phase-C read per qi is one 768B-elem DMA.
    a2a_in = nc.dram_tensor("a2a_in", [4, N_CORES, 128, 6, 64], BF16)
    a2a_out = nc.dram_tensor("a2a_out", [4, N_CORES, 128, 6, 64], BF16)

    with tile.TileContext(nc) as tc:
        with tc.tile_pool(name="persist", bufs=1) as pp:
            cosq = pp.tile([128, c.LOC], BF16, name="cosq")
            sinq = pp.tile([128, c.LOC], BF16, name="sinq")
            cosk = pp.tile([64, c.LOC], BF16, name="cosk")
            sink = pp.tile([64, c.LOC], BF16, name="sink")
            rot_sb = pp.tile([128, 128], F32R, name="rot_sb")
            ones_sb = pp.tile([128, 128], F32R, name="ones_sb")
            hTh = pp.tile([128, c.DP, 2, c.LOC], FP8, name="hTh")
            hTl = pp.tile([128, c.DP, 2, c.LOC], FP8, name="hTl")
            nc.sync.dma_start(hTh[:], hTh_d.ap())
            nc.sync.dma_start(hTl[:], hTl_d.ap())
            for t, d in ((ones_sb, ones_d), (rot_sb, rot_d), (cosk, cosk_d),
                         (sink, sink_d), (cosq, cosq_d), (sinq, sinq_d)):
                nc.scalar.dma_start(t[:], d.ap())
            ones_col_f = ones_sb[:, 0:1]
            ones_row_f = ones_sb[0:1, :]
            ones_col_b = pp.tile([128, 1], BF16, name="ones_col_b")
            nc.vector.memset(ones_col_b[:], 1.0)
            woh_sb = pp.tile([128, 32, 2, 2, 128], FP8, name="woh_sb")
            wol_sb = pp.tile([128, 32, 2, 2, 128], FP8, name="wol_sb")
            mask_sb = pp.tile([128, 4, 512], BF16, name="mask_sb")
            # krope2 memset here so the gather-read into it isn't gated on
            # DVE reaching the end of phase Q.
            krope2 = [pp.tile([128, c.S], BF16, name=f"krope2_{par}")
                      for par in range(2)]
            nc.vector.memset(krope2[0][:], 0.0)
            nc.vector.memset(krope2[1][:], 0.0)

            # small side pool spanning phases A+Q so the first two q b-proj
            # weight loads can slot into the tail of the wa DMA stream.
            pQw0_cm = tc.tile_pool(name="pQ_w0", bufs=2)
            pQw0 = pQw0_cm.__enter__()

            def load_wqb(g, pool):
                w = pool.tile([128, 4, 2, c.QP, 2, 128], FP8, name="wqb_sb")
                nc.sync.dma_start(
                    w[:].rearrange("p o a k i x -> p o (a k i x)"),
                    wqb_d.ap()[g]
                    .rearrange("p o a k i x -> p o (a k i x)"))
                return w

            wq_tiles = {}

            # ================= phase A: local a-projections ===============
            with tc.tile_pool(name="pA_w", bufs=4) as pAw, \
                 tc.tile_pool(name="pA_ev", bufs=4) as pAe, \
                 tc.tile_pool(name="pA_keep", bufs=1) as pAk, \
                 tc.tile_pool(name="pA_ps", bufs=2, space="PSUM") as psA, \
                 tc.tile_pool(name="pA_aux", bufs=1, space="PSUM") as psX, \
                 tc.tile_pool(name="pA_ps1", bufs=1, space="PSUM") as psA1:

                def aproj(m, w_sb, planes):
                    """fp8 hi/lo a-proj for chunk m -> psum [128, LOC]."""
                    ps = psA.tile([128, c.LOC], F32, name="psA")
                    terms = [(0, hTh), (0, hTl)] + ([(1, hTh)] if planes == 2
                                                   else [])
                    i = 0
                    n = len(terms) * c.DP
                    for (pl, h) in terms:
                        for kp in range(c.DP):
                            nc.tensor.matmul(
                                ps[:], w_sb[:, pl, kp, :, :], h[:, kp, :, :],
                                start=(i == 0), stop=(i == n - 1),
                                perf_mode=DR)
                            i += 1
                    return ps

                def load_wa(m):
                    planes = 2
                    w = pAw.tile([128, 2, c.DP, 2, 128], FP8, name="wa_sb")
                    nc.sync.dma_start(
                        w[:, 0:planes].rearrange("p a k i x -> p a (k i x)"),
                        wa_d.ap()[m][:, 0:planes]
                        .rearrange("p a k i x -> p a (k i x)"))
                    return w

                morder = list(range(c.QRCH, c.AM)) + list(range(c.QRCH))
                wa_tiles = {}
                for mm in morder[:3]:
                    wa_tiles[mm] = load_wa(mm)

                def get_wa(idx):
                    m = morder[idx]
                    if idx + 3 < len(morder):
                        wa_tiles[morder[idx + 3]] = load_wa(morder[idx + 3])
                    return wa_tiles.pop(m)

                # ---- ckv chunks (m=12..15) + rope key (m=16) ----
                ssc = psA1.tile([1, c.LOC], F32, name="ss_ps")
                c_ev = []
                for mc in range(c.KVCH):
                    ps = aproj(c.QRCH + mc, get_wa(mc), 2)
                    ev = pAk.tile([128, c.LOC], F32, name=f"c_ev{mc}")
                    nc.scalar.activation(ev[:], ps[:], AF.Copy)
                    c_ev.append(ev)
                    x2 = pAe.tile([128, c.LOC], F32R, name="x2")
                    nc.vector.tensor_mul(x2[:], ev[:], ev[:])
                    nc.tensor.matmul(ssc[:], ones_col_f, x2[:],
                                     start=(mc == 0), stop=(mc == c.KVCH - 1))
                # rms + normalized ckv hi/lo + its gather, ASAP
                t1 = pAe.tile([1, c.LOC], F32, name="rms_t")
                nc.vector.tensor_scalar(
                    t1[:], ssc[:], 1.0 / c.KVR, SA * SA * EPS,
                    mybir.AluOpType.mult, mybir.AluOpType.add)
                st = pAe.tile([1, c.LOC], F32, name="rms_st")
                nc.scalar.activation(st[:], t1[:], AF.Sqrt)
                rsc = pAe.tile([1, c.LOC], F32R, name="rsc")
                with nc.allow_low_precision(reason="f32r for PE broadcast"):
                    nc.vector.reciprocal(rsc[:], st[:])
                bc_ps = psX.tile([128, c.LOC], F32, name="aux_ps")
                nc.tensor.matmul(bc_ps[:], ones_row_f, rsc[:])
                bcc = pAe.tile([128, c.LOC], F32, name="bcc")
                nc.vector.tensor_copy(bcc[:], bc_ps[:])
                cnh = pAk.tile([128, c.KVCH, c.LOC], FP8, name="cnh")
                cnl = pAk.tile([128, c.KVCH, c.LOC], FP8, name="cnl")
                for mc in range(c.KVCH):
                    cnf = pAe.tile([128, c.LOC], F32, name="cnf")
                    nc.vector.tensor_mul(cnf[:], c_ev[mc][:], bcc[:])
                    nc.gpsimd.tensor_copy(cnh[:, mc, :], cnf[:])
                    dfc = pAe.tile([128, c.LOC], F32, name="dfc")
                    nc.gpsimd.tensor_sub(dfc[:], cnf[:], cnh[:, mc, :])
                    nc.gpsimd.tensor_copy(cnl[:, mc, :], dfc[:])
                nc.gpsimd.dma_start(
                    gin_c.ap()[0].rearrange("k p x -> p k x"), cnh[:])
                nc.gpsimd.dma_start(
                    gin_c.ap()[1].rearrange("k p x -> p k x"), cnl[:])

                # ---- collective 1a: AllGather normalized ckv (fp8 hi/lo) --
                nc.gpsimd.collective_compute(
                    "AllGather", mybir.AluOpType.bypass, replica_groups=GRP,
                    ins=[gin_c.ap()], outs=[gout_c.ap()])

                # rope key chunk + its (small) gather
                ps = aproj(c.AM - 1, get_wa(c.KVCH), 2)
                kr = pAe.tile([64, c.LOC], F32R, name="kr")
                nc.scalar.activation(kr[:], ps[0:64, :], AF.Copy)
                rps = psX.tile([128, c.LOC], F32, name="aux_ps")
                nc.tensor.matmul(rps[0:64, :], rot_sb[0:64, 0:64], kr[:])
                rk = pAe.tile([64, c.LOC], F32, name="rk")
                nc.vector.tensor_copy(rk[:], rps[0:64, :])
                ra = pAe.tile([64, c.LOC], F32, name="ra")
                rb = pAe.tile([64, c.LOC], F32, name="rb")
                nc.vector.tensor_mul(ra[:], kr[:], cosk[:])
                nc.vector.tensor_mul(rb[:], rk[:], sink[:])
                kro = pAe.tile([64, c.LOC], BF16, name="kro")
                nc.vector.tensor_add(kro[:], ra[:], rb[:])
                nc.gpsimd.dma_start(gin_k.ap(), kro[:])

                # ---- collective 1b: AllGather rope key ----
                nc.gpsimd.collective_compute(
                    "AllGather", mybir.AluOpType.bypass, replica_groups=GRP,
                    ins=[gin_k.ap()], outs=[gout_k.ap()])

                # ---- qa chunks (m=0..11): raw hi/lo, rms applied later --
                ssq = psA1.tile([1, c.LOC], F32, name="ss_ps")
                qah = pp.tile([128, c.QP, 2, c.LOC], FP8, name="qah")
                qal = pp.tile([128, c.QP, 2, c.LOC], FP8, name="qal")
                for m in range(c.QRCH):
                    if m in (9, 10):
                        wq_tiles[m - 9] = load_wqb(m - 9, pQw0)
                    ps = aproj(m, get_wa(c.KVCH + 1 + m), 2)
                    ev = pAe.tile([128, c.LOC], F32, name="qa_ev")
                    nc.scalar.activation(ev[:], ps[:], AF.Copy, scale=1.0 / SA)
                    x2 = pAe.tile([128, c.LOC], F32R, name="x2")
                    nc.vector.tensor_mul(x2[:], ev[:], ev[:])
                    nc.tensor.matmul(ssq[:], ones_col_f, x2[:],
                                     start=(m == 0), stop=(m == c.QRCH - 1))
                    dst_h = qah[:, m // 2, m % 2, :]
                    nc.vector.tensor_copy(dst_h, ev[:])
                    df = pAe.tile([128, c.LOC], F32, name="df")
                    nc.vector.tensor_sub(df[:], ev[:], dst_h)
                    nc.vector.tensor_copy(qal[:, m // 2, m % 2, :], df[:])
                t2 = pAe.tile([1, c.LOC], F32, name="rms_t2")
                nc.vector.tensor_scalar(
                    t2[:], ssq[:], 1.0 / c.QR, EPS,
                    mybir.AluOpType.mult, mybir.AluOpType.add)
                st2 = pAe.tile([1, c.LOC], F32, name="rms_st2")
                nc.scalar.activation(st2[:], t2[:], AF.Sqrt)
                rsq = pAe.tile([1, c.LOC], F32R, name="rsq")
                with nc.allow_low_precision(reason="f32r for PE broadcast"):
                    nc.vector.reciprocal(rsq[:], st2[:])
                bq_ps = psX.tile([128, c.LOC], F32, name="aux_ps")
                nc.tensor.matmul(bq_ps[:], ones_row_f, rsq[:])
                bcq = pp.tile([128, c.LOC], F32, name="bcq")
                nc.vector.tensor_copy(bcq[:], bq_ps[:])
                cosqn = pp.tile([128, c.LOC], F32, name="cosqn")
                sinqn = pp.tile([128, c.LOC], F32, name="sinqn")
                nc.vector.tensor_mul(cosqn[:], cosq[:], bcq[:])
                nc.vector.tensor_mul(sinqn[:], sinq[:], bcq[:])

            # ================= phase Q: local q b-proj (all heads) ========
            with tc.tile_pool(name="pQ_w", bufs=4) as pQw, \
                 tc.tile_pool(name="pQ_ev", bufs=3) as pQe, \
                 tc.tile_pool(name="pQ_ps", bufs=3, space="PSUM") as psQ, \
                 tc.tile_pool(name="pQ_rot", bufs=1, space="PSUM") as psQr:

                for gg in range(2, 5):
                    wq_tiles[gg] = load_wqb(gg, pQw)

                for g in range(12):
                    if g + 5 < 12:
                        wq_tiles[g + 5] = load_wqb(g + 5, pQw)
                    wq = wq_tiles.pop(g)
                    qn4 = None
                    qr2 = None
                    if g < 8:
                        qn4 = pQe.tile([128, 4, 4, 64], BF16, name="qn4")
                    else:
                        qr2 = [pQe.tile([128, 4, 2, 64], BF16,
                                        name=f"qr2_{x}") for x in range(2)]
                    for oo in range(4):
                        o = 4 * g + oo
                        ps = psQ.tile([128, c.LOC], F32, name="psQ")
                        i = 0
                        for (pl, q) in ((0, qah), (0, qal), (1, qah)):
                            for kp in range(c.QP):
                                nc.tensor.matmul(
                                    ps[:], wq[:, oo, pl, kp, :, :],
                                    q[:, kp, :, :],
                                    start=(i == 0), stop=(i == 3 * c.QP - 1),
                                    perf_mode=DR)
                                i += 1
                        if o < c.H:
                            qt = pQe.tile([128, c.LOC], F32, name="qt")
                            nc.scalar.activation(
                                qt[:], ps[:], AF.Copy, scale=1.0 / SQ)
                            nc.vector.tensor_mul(
                                qn4[:, :, oo, :],
                                qt[:].rearrange("p (r x) -> p r x", r=4),
                                bcq[:].rearrange("p (r x) -> p r x", r=4))
                        else:
                            j = o - c.H
                            ro = pQe.tile([128, c.LOC], F32R, name="ro")
                            nc.scalar.activation(
                                ro[:], ps[:], AF.Copy, scale=1.0 / SQ)
                            rps = psQr.tile([128, c.LOC], F32, name="rpsQ")
                            nc.tensor.matmul(rps[:], rot_sb[:], ro[:])
                            rk = pQe.tile([128, c.LOC], F32, name="qrk")
                            nc.vector.tensor_copy(rk[:], rps[:])
                            a = pQe.tile([128, c.LOC], F32, name="qra")
                            b = pQe.tile([128, c.LOC], F32, name="qrb")
                            nc.vector.tensor_mul(a[:], ro[:], cosqn[:])
                            nc.vector.tensor_mul(b[:], rk[:], sinqn[:])
                            nc.vector.tensor_add(
                                qr2[oo // 2][:, :, oo % 2, :],
                                a[:].rearrange("p (r x) -> p r x", r=4),
                                b[:].rearrange("p (r x) -> p r x", r=4))
                    if g < 8:
                        nc.sync.dma_start(
                            a2a_in.ap()[:, g, :, 0:4, :]
                            .rearrange("r p o x -> p r o x"),
                            qn4[:])
                    else:
                        for x in range(2):
                            d = (g - 8) * 2 + x
                            nc.sync.dma_start(
                                a2a_in.ap()[:, d, :, 4:6, :]
                                .rearrange("r p s x -> p r s x"),
                                qr2[x][:])

            pQw0_cm.__exit__(None, None, None)

            # ================= shared B/C residents =======================
            # gout reads issue BEFORE the A2A issues: they only wait on the
            # AllGather, so phase B's inputs land while phase Q streams.
            pBC_cm = tc.tile_pool(name="pBC", bufs=1)
            pBC = pBC_cm.__enter__()
            pB_cm = tc.tile_pool(name="pB", bufs=1)
            pB = pB_cm.__enter__()
            knopeT = [pBC.tile([128, c.S], BF16, name=f"knopeT_{m}")
                      for m in range(c.HPC)]
            v_sb = [pBC.tile([128, c.HPC * c.VD], BF16, name=f"v_sb_{ki}")
                    for ki in range(c.NKI)]
            wkbv_sb = pB.tile([128, 2, 2, 2, 2, 512], FP8, name="wkbv_sb")
            nc.gpsimd.dma_start(
                wkbv_sb[:].rearrange("p a b i s x -> p (a b i s x)"),
                wkbv_d.ap().rearrange("p a b i s x -> p (a b i s x)"))
            c8 = []
            for pl in range(2):
                t = pB.tile([128, c.KVCH, c.S], FP8, name=f"c8_{pl}")
                for kc in range(c.KVCH):
                    nc.gpsimd.dma_start(
                        t[:, kc, :].rearrange("p (g s x) -> p g s x",
                                              g=4, s=N_CORES),
                        gout_c.ap()[:, pl, kc]
                        .rearrange("s p (g x) -> p g s x", g=4))
                c8.append(t)
            for par in range(2):
                nc.gpsimd.dma_start(
                    krope2[par][64 * par:64 * par + 64, :]
                    .rearrange("p (g s x) -> p g s x", g=4, s=N_CORES),
                    gout_k.ap().rearrange("s p (g x) -> p g s x", g=4))

            # ---- collectives 2..5: AllToAll q^T per 512-col round ----
            for r in range(4):
                nc.gpsimd.collective_compute(
                    "AllToAll", mybir.AluOpType.bypass, replica_groups=GRP,
                    ins=[a2a_in.ap()[r]], outs=[a2a_out.ap()[r]])

            # ================= phase B: kv b-projection (fp8 3-term) ======
            BTERMS = ((0, 0), (1, 0), (0, 1))  # (c plane, w plane)
            with tc.tile_pool(name="pB_ps", bufs=3, space="PSUM") as psB:
                for m in range(c.HPC):
                    for n in range(c.S // 512):
                        ps = psB.tile([128, 512], F32, name="psB")
                        i = 0
                        for (pc, pw) in BTERMS:
                            for kp in range(2):
                                nc.tensor.matmul(
                                    ps[:],
                                    wkbv_sb[:, pw, kp, :, 0,
                                            m * 128:(m + 1) * 128],
                                    c8[pc][:, kp * 2:kp * 2 + 2,
                                           n * 512:(n + 1) * 512],
                                    start=(i == 0), stop=(i == 5),
                                    perf_mode=DR)
                                i += 1
                        nc.scalar.activation(
                            knopeT[m][:, n * 512:(n + 1) * 512], ps[:],
                            AF.Copy, scale=1.0 / SKV)
                for ki in range(c.NKI):
                    ps = psB.tile([128, c.HPC * c.VD], F32, name="psB")
                    i = 0
                    for (pc, pw) in BTERMS:
                        for kp in range(2):
                            nc.tensor.matmul(
                                ps[:],
                                c8[pc][:, kp * 2:kp * 2 + 2,
                                       ki * 128:(ki + 1) * 128],
                                wkbv_sb[:, pw, kp, :, 1, :],
                                start=(i == 0), stop=(i == 5),
                                perf_mode=DR)
                            i += 1
                    nc.scalar.activation(v_sb[ki][:], ps[:], AF.Copy,
                                         scale=1.0 / SKV)
            pB_cm.__exit__(None, None, None)

            # ================= phase C: attention + out-proj ==============
            with tc.tile_pool(name="pC2", bufs=2) as pC2, \
                 tc.tile_pool(name="pCe", bufs=3) as pCe, \
                 tc.tile_pool(name="pCx", bufs=6) as pCx, \
                 tc.tile_pool(name="pCacc", bufs=2) as pCa, \
                 tc.tile_pool(name="pC_mm", bufs=3, space="PSUM") as psM, \
                 tc.tile_pool(name="pC_sT", bufs=2, space="PSUM") as psT, \
                 tc.tile_pool(name="pC_oT", bufs=2, space="PSUM") as psO, \
                 tc.tile_pool(name="pC_den", bufs=1, space="PSUM") as psD:
                # out-proj weights + causal mask needed only here; SP reaches
                # this point right after the Q weight stream, so these land in
                # the idle DMA window before the first a2a_out arrives.
                nc.sync.dma_start(
                    mask_sb[:].rearrange("p j x -> p (j x)"),
                    mask_d.ap().rearrange("p j x -> p (j x)"))
                nc.sync.dma_start(
                    woh_sb[:].rearrange("p m a b x -> p (m a b x)"),
                    woh_d.ap().rearrange("p m a b x -> p (m a b x)"))
                nc.sync.dma_start(
                    wol_sb[:].rearrange("p m a b x -> p (m a b x)"),
                    wol_d.ap().rearrange("p m a b x -> p (m a b x)"))
                for qi in range(c.NQT):
                    q0 = qi * 512
                    q_all = pC2.tile([128, N_CORES, 6, 64], BF16,
                                     name="q_all")
                    nc.sync.dma_start(
                        q_all[:],
                        a2a_out.ap()[qi].rearrange("s p o x -> p s o x"))
                    qnopeT = [q_all[:, :, h, :] for h in range(c.HPC)]
                    qropeT = [q_all[:, :, 4 + j, :] for j in range(2)]

                    oT8 = [[pC2.tile([128, 2, 512], FP8, name=f"o{x}_{pr}")
                            for pr in range(2)] for x in range(2)]
                    nki = 4 * (qi + 1)

                    def attn_head(h):
                        """ki loop for head h: softmax numerator into PSUM,
                        denominator accumulated on DVE in bf16."""
                        oT_ps = psO.tile([128, 512], F32, name="psO")
                        acc = pCa.tile([128, 512], BF16, name="den_acc")
                        for ki in range(nki):
                            sT_ps = psT.tile([128, 512], F32, name="psT")
                            nc.tensor.matmul(
                                sT_ps[:],
                                knopeT[h][:, ki * 128:(ki + 1) * 128],
                                qnopeT[h], start=True, stop=False)
                            nc.tensor.matmul(
                                sT_ps[:],
                                krope2[h % 2][:, ki * 128:(ki + 1) * 128],
                                qropeT[h // 2], start=False, stop=True)
                            ex = pCx.tile([128, 512], BF16, name="expT")
                            nc.scalar.activation(ex[:], sT_ps[:], AF.Exp)
                            jj = ki - (nki - 4)
                            if jj >= 0:
                                nc.vector.tensor_mul(ex[:], ex[:],
                                                     mask_sb[:, jj, :])
                            if ki == 0:
                                nc.vector.tensor_copy(acc[:], ex[:])
                            else:
                                nc.vector.tensor_add(acc[:], acc[:], ex[:])
                            nc.tensor.matmul(
                                oT_ps[:],
                                v_sb[ki][:, h * c.VD:(h + 1) * c.VD],
                                ex[:], start=(ki == 0), stop=(ki == nki - 1))
                        den_ps = psD.tile([1, 512], F32, name="psD")
                        nc.tensor.matmul(den_ps[:], ones_col_b[:], acc[:],
                                         start=True, stop=True)
                        rec = pCe.tile([1, 512], F32R, name="rec")
                        with nc.allow_low_precision(reason="f32r broadcast"):
                            nc.vector.reciprocal(rec[:], den_ps[:])
                        return oT_ps, rec

                    def norm_head(h, oT_ps, rec):
                        """normalize + fp8 hi/lo split of head h's output.
                        The two fp8 casts run on gpsimd (otherwise idle) to
                        keep DVE off the attention critical path."""
                        bc_ps = psM.tile([128, 512], F32, name="psm")
                        nc.tensor.matmul(bc_ps[:], ones_row_f, rec[:])
                        ov = pCe.tile([128, 512], F32, name="ov")
                        nc.vector.tensor_mul(ov[:], oT_ps[:], bc_ps[:])
                        dst_h = oT8[0][h // 2][:, h % 2, :]
                        nc.gpsimd.tensor_copy(dst_h, ov[:])
                        df = pCe.tile([128, 512], F32, name="odf")
                        nc.vector.tensor_sub(df[:], ov[:], dst_h)
                        nc.gpsimd.tensor_copy(oT8[1][h // 2][:, h % 2, :],
                                              df[:])

                    # software-pipeline: head h's attention runs while head
                    # h-1's reciprocal lands, so the broadcast matmul never
                    # stalls PE.
                    prev = None
                    for h in range(c.HPC):
                        cur = attn_head(h)
                        if prev is not None:
                            norm_head(h - 1, *prev)
                        prev = cur
                    norm_head(c.HPC - 1, *prev)

                    for m4 in range(8):
                        ob4 = pCe.tile([128, 4, 512], BF16, name="ob4")
                        for mm in range(4):
                            m = 4 * m4 + mm
                            ps = psM.tile([128, 512], F32, name="psm")
                            i = 0
                            for pr in range(2):
                                for (w, o8) in ((woh_sb, oT8[0][pr]),
                                                (woh_sb, oT8[1][pr]),
                                                (wol_sb, oT8[0][pr])):
                                    nc.tensor.matmul(
                                        ps[:], w[:, m, pr, :, :], o8[:],
                                        start=(i == 0), stop=(i == 5),
                                        perf_mode=DR)
                                    i += 1
                            nc.scalar.activation(
                                ob4[:, mm, :], ps[:], AF.Copy,
                                scale=1.0 / SWO)
                        nc.scalar.dma_start(
                            out_d.ap()[4 * m4:4 * m4 + 4, :, q0:q0 + 512]
                            .rearrange("m p x -> p m x"), ob4[:])
            pBC_cm.__exit__(None, None, None)
    nc.compile()
    return nc


# --------------------------------------------------------------------------
# public entry point
# --------------------------------------------------------------------------

_CACHED = {}


def _get_nc(cfg):
    if cfg not in _CACHED:
        _CACHED[cfg] = build(cfg)
    return _CACHED[cfg]


def kernel(hidden_states, Wq_a, q_a_ln_w, Wq_b, Wkv_a, kv_a_ln_w, Wkv_b, Wo):
    cfg = FULL
    in_maps = prep_inputs(cfg, hidden_states, Wq_a, q_a_ln_w, Wq_b, Wkv_a,
                          kv_a_ln_w, Wkv_b, Wo)
    nc = _get_nc(cfg)
    res = run_bass_kernel_spmd(nc, in_maps, core_ids=list(range(N_CORES)))
    acc = np.zeros((32, 128, cfg.S), np.float32)
    for r in res.results:
        acc += np.asarray(r["outT"], np.float32)
    out = acc.reshape(cfg.D, cfg.S).T
    return np.ascontiguousarray(out).reshape(1, cfg.S, cfg.D)

